# revision 8
# baseline (speedup 1.0000x reference)
"""DCT-blur kernel for 8 Trainium2 NeuronCores.

Computes, per image X [256,256]:
    out = C^T @ (M_b (*) (C @ X @ C^T)) @ C
where C is the orthonormal DCT-II matrix and M_b a per-batch-item
frequency fade mask derived from t[b]:
    sigma = exp(log(.5)(1-t) + log(20)t); tau = sigma^2/2
    fade[i,j] = exp(-(f_i^2+f_j^2) tau);  fade<0.01 -> 0
    M = fade*(1-0.001) + 0.001,   f_i = pi*i/256
Since C is orthonormal the 0.001 floor is pulled out exactly:
    out = 0.001*X + 0.999 * C^T @ (fade_clamped (*) (C X C^T)) @ C

The clamped fade has quarter-disk support with axis cutoff s(t) =
O(40^-t) indices.  Per batch item the cutoff (rounded up to 32) is
baked into the program:
  * s <= 256 (~96% of items): bf16 path whose four matmul stages touch
    only the first s frequencies, in 128-blocks (kb = ceil(s/128)).
    bf16 runs any free dim at 1 cycle/row (fp32r needs >=256) and its
    128-wide weights get fast-weight-load, so LDWEIGHTS hides.
  * s > 256: full 16-matmul fp32r path (minimal blur; these items
    dominate the output absmax so they keep the precise dtype).

Numerics vs the 2e-2 gate: bf16 output rounding ~2e-3; bf16 operand
rounding on partially-blurred items ~3e-3 (mask-filtered for small s);
for s<=128 the fade<0.01 threshold is skipped (each borderline
coefficient spreads as a cos wave of amplitude <=0.01*|Y|*2/256, total
~1e-4); slots whose output is evicted on the Scalar engine drop the
0.001*X term (~1e-3).  Measured total ~3-5e-3.

Layout: images live per-partition as row pairs (partition p holds rows
2p, 2p+1), so every x / y DMA is a straight [128, 3*512] copy (2KB
lines, one dma_start per 3-channel slot).  The DCT constants absorb the
permutation: stage-1 rhs ctp is C^T row-split even/odd, stage-4 weights
cm4 are C column-split even/odd, making stage-4 PSUM output land
exactly in row-pair layout, elementwise-aligned with the x tile for the
final + 0.001*X.

Engine budget: every PSUM eviction runs at 1 elem/cycle/lane (fp32
PSUM source), so the ~70k eviction columns are split between ACT and
DVE by a greedy ns-estimate balancer; the mask multiply doubles as the
stage-2 eviction and the 0.001*X add as stage-4's (DVE only).  Heavy
(full/medium) and light slots are interleaved so the PE never idles
long enough to re-throttle (HAM) and eviction load stays even; the
first and last slots are the sparsest.  A few junk matmuls plus
prebuilt masks warm the PE during the input-DMA head.

Sharding: pure data parallel, batch 128 -> 16 slots per core.  The
host sorts items by cutoff (descending), deals round-robin, and each
slot's config is the max over its 8 cores' items, so one SPMD program
serves all cores.
"""

from contextlib import ExitStack

import numpy as np
import ml_dtypes

import concourse.bass as bass
import concourse.tile as tile
from concourse import bacc, mybir
from concourse.bass_utils import run_bass_kernel_spmd

B, CH, N = 128, 3, 256
NCORES = 8
BPC = B // NCORES  # batch items (slots) per core
H = N // 2  # 128 = partition count
W_IMG = 2 * N  # 512 floats per partition per image (row pair)

MIN_BLUR, MAX_BLUR, MIN_SCALE = 0.5, 20.0, 0.001

F32 = mybir.dt.float32
F32R = mybir.dt.float32r
BF16 = mybir.dt.bfloat16
BF16_NP = ml_dtypes.bfloat16
ALU = mybir.AluOpType
ACTF = mybir.ActivationFunctionType

FULL = 1000  # cfg value marking the fp32r full path (s > 256)

TAU_SCALE = float(2.0 * np.log(MAX_BLUR / MIN_BLUR))
TAU_BIAS = float(np.log(0.5 * MIN_BLUR * MIN_BLUR))
THR = float(np.float32(np.float32(1.0 - MIN_SCALE) * np.float32(0.01)))


def _proc_order(cfg):
    """Interleave heavy (full/medium) and light slots; start and end light.
    fp32r (FULL) slots go mid-sequence so their fp32 constants (DMA'd after
    the bf16 ones) have landed."""
    n_b = len(cfg)
    full = [b for b in range(n_b) if cfg[b] == FULL]
    med = [b for b in range(n_b) if 128 < cfg[b] < FULL]
    heavy = med[:2] + full + med[2:]
    light = [b for b in range(n_b) if cfg[b] <= 128][::-1]  # sparsest first
    if not light:
        return heavy
    order = [light[0]]
    li = 1
    for h in heavy:
        order.append(h)
        if li < len(light):
            order.append(light[li])
            li += 1
    order += light[li:]
    return order


def build_nc(cfg):
    """cfg: per-slot axis cutoff in dealt order; FULL = fp32r path, else
    s in {32,64,...,256} = bf16 block path."""
    n_b = len(cfg)
    full_slots = [b for b in range(n_b) if cfg[b] == FULL]
    bf_slots = [b for b in range(n_b) if cfg[b] != FULL]
    n32, n16 = len(full_slots), len(bf_slots)
    x32_idx = {b: i for i, b in enumerate(full_slots)}
    x16_idx = {b: i for i, b in enumerate(bf_slots)}
    procorder = _proc_order(cfg)

    nc = bacc.Bacc(
        "TRN2",
        target_bir_lowering=False,
        debug=False,
        num_devices=NCORES,
    )
    if n32:
        x32_d = nc.dram_tensor("x32", [n32, H, CH * W_IMG], F32R, kind="ExternalInput").ap()
        ctp32_d = nc.dram_tensor("ctp32", [H, 2 * N], F32R, kind="ExternalInput").ap()
        cmt32_d = nc.dram_tensor("cmt32", [H, 2 * N], F32R, kind="ExternalInput").ap()
        cm32_d = nc.dram_tensor("cm32", [H, 2 * N], F32R, kind="ExternalInput").ap()
        cm432_d = nc.dram_tensor("cm432", [H, 2 * N], F32R, kind="ExternalInput").ap()
    if n16:
        x16_d = nc.dram_tensor("x16", [n16, H, CH * W_IMG], BF16, kind="ExternalInput").ap()
        ctp16_d = nc.dram_tensor("ctp16", [H, 2 * N], BF16, kind="ExternalInput").ap()
        cmt16_d = nc.dram_tensor("cmt16", [H, 2 * N], BF16, kind="ExternalInput").ap()
        cm16_d = nc.dram_tensor("cm16", [H, 2 * N], BF16, kind="ExternalInput").ap()
        cm416_d = nc.dram_tensor("cm416", [H, 2 * N], BF16, kind="ExternalInput").ap()
    t_d = nc.dram_tensor("t", [1, n_b], F32, kind="ExternalInput").ap()
    f2_d = nc.dram_tensor("f2", [1, N], F32, kind="ExternalInput").ap()
    y_d = nc.dram_tensor("y", [n_b, H, CH * W_IMG], BF16, kind="ExternalOutput").ap()

    # greedy ACT/DVE eviction balancer (ns estimates incl. fixed overhead)
    load = {"act": 0.0, "dve": 0.0}

    def pick(cols):
        a = load["act"] + 0.85 * cols + 250
        v = load["dve"] + 1.10 * cols + 250
        if a <= v:
            load["act"] = a
            return "act"
        load["dve"] = v
        return "dve"

    with tile.TileContext(nc) as tc, ExitStack() as ctx:
        cpool = ctx.enter_context(tc.tile_pool(name="consts", bufs=1))
        wpool = ctx.enter_context(tc.tile_pool(name="work", bufs=2))
        ppool = ctx.enter_context(tc.tile_pool(name="psum", bufs=2, space="PSUM"))

        def evict(dst, src, cols, eng):
            if eng == "act":
                nc.scalar.copy(dst, src)
            else:
                nc.vector.tensor_copy(dst, src)

        # ---- small constants first ----
        t_sb = cpool.tile([1, n_b], F32, tag="t", name="t_sb")
        nc.sync.dma_start(t_sb, t_d)
        f2_sb = cpool.tile([1, N], F32, tag="f2", name="f2_sb")
        nc.sync.dma_start(f2_sb, f2_d)
        if n16:
            ctp16 = cpool.tile([H, 2 * N], BF16, tag="ctp16", name="ctp16")
            nc.sync.dma_start(ctp16, ctp16_d)
            cmt16 = cpool.tile([H, 2 * N], BF16, tag="cmt16", name="cmt16")
            nc.sync.dma_start(cmt16, cmt16_d)
            cm16 = cpool.tile([H, 2 * N], BF16, tag="cm16", name="cm16")
            nc.sync.dma_start(cm16, cm16_d)
            cm416 = cpool.tile([H, 2 * N], BF16, tag="cm416", name="cm416")
            nc.sync.dma_start(cm416, cm416_d)

        # ---- hoisted x DMAs for the first processed bf16 slots ----
        xt = {}
        for b in procorder[:3]:
            if cfg[b] == FULL:
                continue
            xs = wpool.tile([H, CH * W_IMG], BF16, tag="x16", bufs=6, name=f"x_{b}")
            nc.sync.dma_start(xs, x16_d[x16_idx[b]])
            xt[b] = xs

        if n32:
            ctp32 = cpool.tile([H, 2 * N], F32R, tag="ctp32", name="ctp32")
            nc.sync.dma_start(ctp32, ctp32_d)
            cmt32 = cpool.tile([H, 2 * N], F32R, tag="cmt32", name="cmt32")
            nc.sync.dma_start(cmt32, cmt32_d)
            cm32 = cpool.tile([H, 2 * N], F32R, tag="cm32", name="cm32")
            nc.sync.dma_start(cm32, cm32_d)
            cm432 = cpool.tile([H, 2 * N], F32R, tag="cm432", name="cm432")
            nc.sync.dma_start(cm432, cm432_d)

        # ---- PE warmup: junk matmuls with no data deps (HAM ramp) ----
        wdt = BF16 if n16 else F32R
        wtile = cpool.tile([H, H], wdt, tag="warm", name="warm")
        nc.vector.memset(wtile, 0.0)
        pj = ppool.tile([H, H], F32, tag="ps2", name="pjunk")
        for _ in range(10):
            nc.tensor.matmul(pj, wtile, wtile, start=True, stop=True)

        # ---- blur schedule ----
        tbias_sb = cpool.tile([1, 1], F32, tag="tbias", name="tbias_sb")
        nc.vector.memset(tbias_sb, TAU_BIAS)
        tau_sb = cpool.tile([1, n_b], F32, tag="tau", name="tau_sb")
        nc.scalar.activation(tau_sb, t_sb, ACTF.Exp, bias=tbias_sb, scale=TAU_SCALE)
        ntau_sb = cpool.tile([1, n_b], F32, tag="ntau", name="ntau_sb")
        nc.vector.tensor_scalar_mul(ntau_sb, tau_sb, -1.0)

        mask_sb = {}  # slot -> mask tile (0.999*fade, thresholded if s>128)

        def build_mask(b):
            full = cfg[b] == FULL
            s = N if full else cfg[b]
            kb = 1 if s <= H else 2
            kw = [min(s, H)] if kb == 1 else [H, s - H]
            u = wpool.tile(
                [1, s], F32R if full else BF16,
                tag="u32" if full else "u16", name=f"u_{b}",
            )
            nc.scalar.activation(
                u, f2_sb[0:1, 0:s], ACTF.Exp, scale=ntau_sb[0:1, b : b + 1]
            )
            psm = ppool.tile([H, kb * s], F32, tag="ps3", name=f"psm_{b}")
            for m2 in range(kb):
                nc.tensor.matmul(
                    psm[0 : kw[m2], m2 * s : m2 * s + s],
                    u[0:1, m2 * H : m2 * H + kw[m2]],
                    u[0:1, 0:s],
                    start=True,
                    stop=True,
                )
            m = wpool.tile([H, kb * s], F32, tag="mask", bufs=n32 + 5, name=f"mask_{b}")
            if kb == 1:
                # no threshold: borderline coeffs spread as ~1e-4 cos waves
                nc.scalar.activation(
                    m[0:s, :], psm[0:s, :], ACTF.Copy, scale=1.0 - MIN_SCALE
                )
            else:
                fade = wpool.tile([H, kb * s], F32, tag="fade", name=f"fade_{b}")
                for m2 in range(kb):
                    sl = (slice(0, kw[m2]), slice(m2 * s, m2 * s + s))
                    nc.scalar.activation(
                        fade[sl], psm[sl], ACTF.Copy, scale=1.0 - MIN_SCALE
                    )
                    nc.vector.scalar_tensor_tensor(
                        m[sl], fade[sl], THR, fade[sl], op0=ALU.is_ge, op1=ALU.mult
                    )
            mask_sb[b] = m

        # prebuild masks: first processed slot + the fp32r slots
        for b in dict.fromkeys(procorder[:1] + full_slots):
            build_mask(b)

        def get_x(b, dt, tensor, idx):
            if b in xt:
                return xt[b]
            xs = wpool.tile(
                [H, CH * W_IMG], dt, tag="x32" if dt is F32R else "x16",
                bufs=2 if dt is F32R else 6, name=f"x_{b}",
            )
            nc.sync.dma_start(xs, tensor[idx])
            return xs

        def emit_bf16(b):
            s = cfg[b]
            kb = 1 if s <= H else 2
            kw = [min(s, H)] if kb == 1 else [H, s - H]
            xs = get_x(b, BF16, x16_d, x16_idx[b])
            if b not in mask_sb:
                build_mask(b)
            s1, s2, s3 = {}, {}, {}
            # stage 1: S1 = (C X)^T -> [w, k<s]; m = w-half, r = row parity
            for j in range(CH):
                p1 = ppool.tile([H, 2 * s], F32, tag="ps1", name=f"p1_{b}_{j}")
                for m in range(2):
                    for r in range(2):
                        nc.tensor.matmul(
                            p1[:, m * s : (m + 1) * s],
                            xs[:, j * W_IMG + r * N + m * H : j * W_IMG + r * N + m * H + H],
                            ctp16[:, r * N : r * N + s],
                            start=(r == 0),
                            stop=(r == 1),
                        )
                t1 = wpool.tile([H, 2 * s], BF16, tag=f"s1x{j}", bufs=3, name=f"s1_{b}_{j}")
                evict(t1, p1, 2 * s, pick(2 * s))
                s1[j] = t1
            # stage 2 + mask: S2T = mask (*) (C X C^T)^T -> [n<s, k<s]
            for j in range(CH):
                p2 = ppool.tile([H, kb * s], F32, tag="ps2", name=f"p2_{b}_{j}")
                for m2 in range(kb):
                    for ww in range(2):
                        nc.tensor.matmul(
                            p2[0 : kw[m2], m2 * s : m2 * s + s],
                            cmt16[:, ww * N + m2 * H : ww * N + m2 * H + kw[m2]],
                            s1[j][:, ww * s : (ww + 1) * s],
                            start=(ww == 0),
                            stop=(ww == 1),
                        )
                t2 = wpool.tile([H, kb * s], BF16, tag=f"s2x{j}", bufs=3, name=f"s2_{b}_{j}")
                load["dve"] += 1.10 * kb * s + 250
                for m2 in range(kb):
                    sl = (slice(0, kw[m2]), slice(m2 * s, m2 * s + s))
                    nc.vector.tensor_mul(t2[sl], p2[sl], mask_sb[b][sl])
                s2[j] = t2
            # stage 3: S3 = S2m @ C -> [k<s, h]
            for j in range(CH):
                p3 = ppool.tile([H, kb * N], F32, tag="ps3", name=f"p3_{b}_{j}")
                for m3 in range(kb):
                    for nn in range(kb):
                        nc.tensor.matmul(
                            p3[0 : kw[m3], m3 * N : m3 * N + N],
                            s2[j][0 : kw[nn], nn * s + m3 * H : nn * s + m3 * H + kw[m3]],
                            cm16[0 : kw[nn], nn * N : nn * N + N],
                            start=(nn == 0),
                            stop=(nn == kb - 1),
                        )
                t3 = wpool.tile([H, kb * N], BF16, tag=f"s3x{j}", bufs=3, name=f"s3_{b}_{j}")
                eng = pick(kb * N)
                for m3 in range(kb):
                    sl = (slice(0, kw[m3]), slice(m3 * N, m3 * N + N))
                    evict(t3[sl], p3[sl], N, eng)
                s3[j] = t3
            # stage 4: Z = C^T @ S3 in row-pair layout; out eviction fused
            o = wpool.tile([H, CH * W_IMG], BF16, tag="o", bufs=3, name=f"o_{b}")
            for j in range(CH):
                p4 = ppool.tile([H, 2 * N], F32, tag="ps4", name=f"p4_{b}_{j}")
                for m in range(2):
                    for kp in range(kb):
                        nc.tensor.matmul(
                            p4[:, m * N : (m + 1) * N],
                            cm416[0 : kw[kp], kp * N + m * H : kp * N + m * H + H],
                            s3[j][0 : kw[kp], kp * N : kp * N + N],
                            start=(kp == 0),
                            stop=(kp == kb - 1),
                        )
                osl = o[:, j * W_IMG : (j + 1) * W_IMG]
                if pick(2 * N) == "dve":
                    nc.vector.scalar_tensor_tensor(
                        osl, xs[:, j * W_IMG : (j + 1) * W_IMG], MIN_SCALE, p4,
                        op0=ALU.mult, op1=ALU.add,
                    )
                else:
                    nc.scalar.copy(osl, p4)  # drops 0.001*X (~1e-3)
            nc.sync.dma_start(y_d[b], o)

        def emit_full(b):
            xs = get_x(b, F32R, x32_d, x32_idx[b])
            s1, s2, s3 = {}, {}, {}
            for j in range(CH):
                p1 = ppool.tile([H, 2 * N], F32, tag="ps1", name=f"p1_{b}_{j}")
                for m in range(2):
                    for r in range(2):
                        nc.tensor.matmul(
                            p1[:, m * N : (m + 1) * N],
                            xs[:, j * W_IMG + r * N + m * H : j * W_IMG + r * N + m * H + H],
                            ctp32[:, r * N : (r + 1) * N],
                            start=(r == 0),
                            stop=(r == 1),
                        )
                t1 = wpool.tile([H, 2 * N], F32R, tag=f"s1d{j}", bufs=3, name=f"s1_{b}_{j}")
                evict(t1, p1, 2 * N, pick(2 * N))
                s1[j] = t1
            for j in range(CH):
                p2 = ppool.tile([H, 2 * N], F32, tag="ps2", name=f"p2_{b}_{j}")
                for m2 in range(2):
                    for ww in range(2):
                        nc.tensor.matmul(
                            p2[:, m2 * N : (m2 + 1) * N],
                            cmt32[:, ww * N + m2 * H : ww * N + m2 * H + H],
                            s1[j][:, ww * N : (ww + 1) * N],
                            start=(ww == 0),
                            stop=(ww == 1),
                        )
                t2 = wpool.tile([H, 2 * N], F32R, tag=f"s2d{j}", bufs=3, name=f"s2_{b}_{j}")
                load["dve"] += 1.10 * 2 * N + 250
                nc.vector.tensor_mul(t2, p2, mask_sb[b])
                s2[j] = t2
            for j in range(CH):
                p3 = ppool.tile([H, 2 * N], F32, tag="ps3", name=f"p3_{b}_{j}")
                for m3 in range(2):
                    for nn in range(2):
                        nc.tensor.matmul(
                            p3[:, m3 * N : (m3 + 1) * N],
                            s2[j][:, nn * N + m3 * H : nn * N + m3 * H + H],
                            cm32[:, nn * N : (nn + 1) * N],
                            start=(nn == 0),
                            stop=(nn == 1),
                        )
                t3 = wpool.tile([H, 2 * N], F32R, tag=f"s3d{j}", bufs=3, name=f"s3_{b}_{j}")
                evict(t3, p3, 2 * N, pick(2 * N))
                s3[j] = t3
            o = wpool.tile([H, CH * W_IMG], BF16, tag="o", bufs=3, name=f"o_{b}")
            for j in range(CH):
                p4 = ppool.tile([H, 2 * N], F32, tag="ps4", name=f"p4_{b}_{j}")
                for m in range(2):
                    for kp in range(2):
                        nc.tensor.matmul(
                            p4[:, m * N : (m + 1) * N],
                            cm432[:, kp * N + m * H : kp * N + m * H + H],
                            s3[j][:, kp * N : (kp + 1) * N],
                            start=(kp == 0),
                            stop=(kp == 1),
                        )
                load["dve"] += 1.10 * 2 * N + 250
                nc.vector.scalar_tensor_tensor(
                    o[:, j * W_IMG : (j + 1) * W_IMG],
                    xs[:, j * W_IMG : (j + 1) * W_IMG],
                    MIN_SCALE,
                    p4,
                    op0=ALU.mult,
                    op1=ALU.add,
                )
            nc.sync.dma_start(y_d[b], o)

        for b in procorder:
            if cfg[b] == FULL:
                emit_full(b)
            else:
                emit_bf16(b)

    nc.compile()
    return nc


def host_constants():
    n = np.arange(N, dtype=np.float64)
    C = np.cos(np.pi * (n[None, :] + 0.5) * n[:, None] / N)
    scale = np.where(n[:, None] == 0, np.sqrt(1.0 / N), np.sqrt(2.0 / N))
    C = (C * scale).astype(np.float32)
    f = (np.pi * np.arange(N) / N).astype(np.float32)
    f2 = (f * f).astype(np.float32)
    A = np.ascontiguousarray(C.T)  # A[h, k] = C[k, h]
    # ctp[p, r*N+k] = C[k, 2p+r]          (stage-1 rhs, rows even/odd split)
    ctp = A.reshape(H, 2, N).reshape(H, 2 * N)
    # cmt[p, ww*N+n] = C[n, ww*128+p]     (stage-2 weights, natural w-halves)
    cmt = A.reshape(2, H, N).transpose(1, 0, 2).reshape(H, 2 * N)
    # cm[p, nn*N+h] = C[nn*128+p, h]      (stage-3 rhs, natural rows)
    cm = C.reshape(2, H, N).transpose(1, 0, 2).reshape(H, 2 * N)
    # cm4[p, kp*N+m*H+w] = C[kp*128+p, 2w+m]  (stage-4 weights, cols split)
    cm4 = C.reshape(2, H, H, 2).transpose(1, 0, 3, 2).reshape(H, 2 * N)
    return (
        np.ascontiguousarray(ctp),
        np.ascontiguousarray(cmt),
        np.ascontiguousarray(cm),
        np.ascontiguousarray(cm4),
        f2,
    )


def s_of_t(t):
    """Per-item axis cutoff: smallest s (with safety margin) such that
    every kept fade entry (>=0.01) has both indices < s."""
    t64 = np.asarray(t, dtype=np.float64)
    sigma = np.exp(np.log(MIN_BLUR) * (1 - t64) + np.log(MAX_BLUR) * t64)
    tau = sigma * sigma / 2.0
    lim = np.log(100.0) / tau  # keep (i,j) with f_i^2+f_j^2 <= lim
    imax = np.floor(N * np.sqrt(lim) / np.pi).astype(np.int64)
    return np.minimum(imax + 3, 2 * N).astype(np.int64)


def interleave(img):
    """[3,256,256] fp32 -> [128, 3*512] row-pair layout."""
    return np.ascontiguousarray(
        img.reshape(CH, H, 2, N).transpose(1, 0, 2, 3).reshape(H, CH * W_IMG)
    )


def deinterleave(arr):
    """[128, 3*512] -> [3,256,256]."""
    return arr.reshape(H, CH, 2, N).transpose(1, 0, 2, 3).reshape(CH, N, N)


_CACHE = {}


def _get_nc(cfg):
    if cfg not in _CACHE:
        _CACHE[cfg] = build_nc(cfg)
    return _CACHE[cfg]


def _run(x, t, trace=False, tmpdir=None):
    x = np.ascontiguousarray(np.asarray(x, dtype=np.float32))
    t = np.asarray(t, dtype=np.float32)
    assert x.shape == (B, CH, N, N) and t.shape == (B,)

    s_item = s_of_t(t)
    # sort densest first, deal round-robin: slot b of core c gets item
    # order[b*8+c]; slot config = group max (sorted -> first of group)
    order = np.argsort(-s_item, kind="stable")
    cfg = []
    for b in range(BPC):
        smax = int(s_item[order[b * NCORES]])
        cfg.append(FULL if smax > N else int(min(N, ((smax + 31) // 32) * 32)))
    cfg = tuple(cfg)
    nc = _get_nc(cfg)

    ctp, cmt, cm, cm4, f2 = host_constants()
    n32 = sum(1 for s in cfg if s == FULL)
    in_maps = []
    for c in range(NCORES):
        items = order[np.arange(BPC) * NCORES + c]  # slot b -> batch index
        x32 = np.empty((n32, H, CH * W_IMG), np.float32) if n32 else None
        x16 = (
            np.empty((BPC - n32, H, CH * W_IMG), BF16_NP) if n32 < BPC else None
        )
        for b in range(BPC):
            il = interleave(x[items[b]])
            if cfg[b] == FULL:
                x32[b] = il
            else:
                x16[b - n32] = il.astype(BF16_NP)
        m = {
            "t": np.ascontiguousarray(t[items].reshape(1, BPC)),
            "f2": f2.reshape(1, N),
        }
        if n32:
            m["x32"] = x32
            m["ctp32"], m["cmt32"], m["cm32"], m["cm432"] = ctp, cmt, cm, cm4
        if n32 < BPC:
            m["x16"] = x16
            m["ctp16"] = ctp.astype(BF16_NP)
            m["cmt16"] = cmt.astype(BF16_NP)
            m["cm16"] = cm.astype(BF16_NP)
            m["cm416"] = cm4.astype(BF16_NP)
        in_maps.append(m)
    res = run_bass_kernel_spmd(
        nc, in_maps, core_ids=list(range(NCORES)), trace=trace, tmpdir=tmpdir
    )
    out = np.empty_like(x)
    for c in range(NCORES):
        items = order[np.arange(BPC) * NCORES + c]
        y = res.results[c]["y"].astype(np.float32)
        for b in range(BPC):
            out[items[b]] = deinterleave(y[b])
    return out, res


def kernel(x, t):
    out, _ = _run(x, t)
    return out


def kernel_with_profile(x, t, tmpdir=None):
    out, res = _run(x, t, trace=True, tmpdir=tmpdir)
    return out, res


# revision 20
# speedup vs baseline: 1.0473x; 1.0473x over previous
"""DCT-blur kernel for 8 Trainium2 NeuronCores.

Computes, per image X [256,256]:
    out = C^T @ (M_b (*) (C @ X @ C^T)) @ C
where C is the orthonormal DCT-II matrix and M_b a per-batch-item
frequency fade mask derived from t[b]:
    sigma = exp(log(.5)(1-t) + log(20)t); tau = sigma^2/2
    fade[i,j] = exp(-(f_i^2+f_j^2) tau);  fade<0.01 -> 0
    M = fade*(1-0.001) + 0.001,   f_i = pi*i/256
Since C is orthonormal the 0.001 floor is pulled out exactly:
    out = 0.001*X + 0.999 * C^T @ (fade_clamped (*) (C X C^T)) @ C

The clamped fade has quarter-disk support with axis cutoff s(t) =
O(40^-t) indices.  Per batch item the cutoff (rounded up to 32) is
baked into the program:
  * s <= 256 (~96% of items): bf16 path whose four matmul stages touch
    only the first s frequencies, in 128-blocks (kb = ceil(s/128)).
    bf16 runs any free dim at 1 cycle/row (fp32r needs >=256) and its
    128-wide weights get fast-weight-load, so LDWEIGHTS hides.
  * s > 256: full 16-matmul fp32r path (minimal blur; these items
    dominate the output absmax so they keep the precise dtype).

Numerics vs the 2e-2 gate: bf16 output rounding ~2e-3; bf16 operand
rounding on partially-blurred items ~3e-3 (mask-filtered for small s);
for s<=128 the fade<0.01 threshold is skipped (each borderline
coefficient spreads as a cos wave of amplitude <=0.01*|Y|*2/256, total
~1e-4); slots whose output is evicted on the Scalar engine drop the
0.001*X term (~1e-3).  Measured total ~3-5e-3.

Layout: images live per-partition as row pairs (partition p holds rows
2p, 2p+1), so every x / y DMA is a straight [128, 3*512] copy (2KB
lines, one dma_start per 3-channel slot).  The DCT constants absorb the
permutation: stage-1 rhs ctp is C^T row-split even/odd, stage-4 weights
cm4 are C column-split even/odd, making stage-4 PSUM output land
exactly in row-pair layout, elementwise-aligned with the x tile for the
final + 0.001*X.

Engine budget: every PSUM eviction runs at 1 elem/cycle/lane (fp32
PSUM source), so the ~70k eviction columns are split between ACT and
DVE by a greedy ns-estimate balancer; the mask multiply doubles as the
stage-2 eviction and the 0.001*X add as stage-4's (DVE only).  Heavy
(full/medium) and light slots are interleaved so the PE never idles
long enough to re-throttle (HAM) and eviction load stays even; the
first and last slots are the sparsest.  A few junk matmuls plus
prebuilt masks warm the PE during the input-DMA head.

Sharding: pure data parallel, batch 128 -> 16 slots per core.  The
host sorts items by cutoff (descending), deals round-robin, and each
slot's config is the max over its 8 cores' items, so one SPMD program
serves all cores.
"""

from contextlib import ExitStack

import numpy as np
import ml_dtypes

import concourse.bass as bass
import concourse.tile as tile
from concourse import bacc, mybir
from concourse.bass_utils import run_bass_kernel_spmd

B, CH, N = 128, 3, 256
NCORES = 8
BPC = B // NCORES  # batch items (slots) per core
H = N // 2  # 128 = partition count
W_IMG = 2 * N  # 512 floats per partition per image (row pair)

MIN_BLUR, MAX_BLUR, MIN_SCALE = 0.5, 20.0, 0.001

F32 = mybir.dt.float32
F32R = mybir.dt.float32r
BF16 = mybir.dt.bfloat16
BF16_NP = ml_dtypes.bfloat16
ALU = mybir.AluOpType
ACTF = mybir.ActivationFunctionType

FULL = 1000  # cfg value marking the fp32r full path (s > 256)

TAU_SCALE = float(2.0 * np.log(MAX_BLUR / MIN_BLUR))
TAU_BIAS = float(np.log(0.5 * MIN_BLUR * MIN_BLUR))
THR = float(np.float32(np.float32(1.0 - MIN_SCALE) * np.float32(0.01)))
HALF_LN999 = float(0.5 * np.log(1.0 - MIN_SCALE))


def _proc_order(cfg):
    """Interleave heavy (full/medium) and light slots; start and end light.
    fp32r (FULL) slots go mid-sequence so their fp32 constants (DMA'd after
    the bf16 ones) have landed."""
    n_b = len(cfg)
    full = [b for b in range(n_b) if cfg[b] == FULL]
    med = [b for b in range(n_b) if 128 < cfg[b] < FULL]
    heavy = med[:2] + full + med[2:]
    light = [b for b in range(n_b) if cfg[b] <= 128][::-1]  # sparsest first
    if not light:
        return heavy
    order = [light[0]]
    li = 1
    for h in heavy:
        order.append(h)
        if li < len(light):
            order.append(light[li])
            li += 1
    order += light[li:]
    return order


def build_nc(cfg):
    """cfg: per-slot axis cutoff in dealt order; FULL = fp32r path, else
    s in {32,64,...,256} = bf16 block path."""
    n_b = len(cfg)
    full_slots = [b for b in range(n_b) if cfg[b] == FULL]
    bf_slots = [b for b in range(n_b) if cfg[b] != FULL]
    n32, n16 = len(full_slots), len(bf_slots)
    x32_idx = {b: i for i, b in enumerate(full_slots)}
    x16_idx = {b: i for i, b in enumerate(bf_slots)}
    procorder = _proc_order(cfg)

    nc = bacc.Bacc(
        "TRN2",
        target_bir_lowering=False,
        debug=False,
        num_devices=NCORES,
    )
    if n32:
        x32_d = nc.dram_tensor("x32", [n32, H, CH * W_IMG], F32R, kind="ExternalInput").ap()
        ctp32_d = nc.dram_tensor("ctp32", [H, 2 * N], F32R, kind="ExternalInput").ap()
        cmt32_d = nc.dram_tensor("cmt32", [H, 2 * N], F32R, kind="ExternalInput").ap()
        cm32_d = nc.dram_tensor("cm32", [H, 2 * N], F32R, kind="ExternalInput").ap()
        cm432_d = nc.dram_tensor("cm432", [H, 2 * N], F32R, kind="ExternalInput").ap()
    if n16:
        x16_d = nc.dram_tensor("x16", [n16, H, CH * W_IMG], BF16, kind="ExternalInput").ap()
        ctp16_d = nc.dram_tensor("ctp16", [H, 2 * N], BF16, kind="ExternalInput").ap()
        cmt16_d = nc.dram_tensor("cmt16", [H, 2 * N], BF16, kind="ExternalInput").ap()
        cm16_d = nc.dram_tensor("cm16", [H, 2 * N], BF16, kind="ExternalInput").ap()
        cm416_d = nc.dram_tensor("cm416", [H, 2 * N], BF16, kind="ExternalInput").ap()
    t_d = nc.dram_tensor("t", [1, n_b], F32, kind="ExternalInput").ap()
    f2_d = nc.dram_tensor("f2", [1, N], F32, kind="ExternalInput").ap()
    t128_d = nc.dram_tensor("t128", [H, n_b], F32, kind="ExternalInput").ap()
    f2t_d = nc.dram_tensor("f2t", [H, 2], F32, kind="ExternalInput").ap()
    y_d = nc.dram_tensor("y", [n_b, H, CH * W_IMG], BF16, kind="ExternalOutput").ap()

    # greedy ACT/DVE eviction balancer (ns estimates incl. fixed overhead)
    load = {"act": 0.0, "dve": 0.0}

    def pick(cols):
        a = load["act"] + 0.85 * cols + 250
        v = load["dve"] + 1.10 * cols + 250
        if a <= v:
            load["act"] = a
            return "act"
        load["dve"] = v
        return "dve"

    with tile.TileContext(nc) as tc, ExitStack() as ctx:
        cpool = ctx.enter_context(tc.tile_pool(name="consts", bufs=1))
        wpool = ctx.enter_context(tc.tile_pool(name="work", bufs=2))
        ppool = ctx.enter_context(tc.tile_pool(name="psum", bufs=2, space="PSUM"))

        def evict(dst, src, cols, eng):
            if eng == "act":
                nc.scalar.copy(dst, src)
            else:
                nc.vector.tensor_copy(dst, src)

        # ---- small constants first ----
        t_sb = cpool.tile([1, n_b], F32, tag="t", name="t_sb")
        nc.sync.dma_start(t_sb, t_d)
        f2_sb = cpool.tile([1, N], F32, tag="f2", name="f2_sb")
        nc.sync.dma_start(f2_sb, f2_d)
        t128_sb = cpool.tile([H, n_b], F32, tag="t128", name="t128_sb")
        nc.sync.dma_start(t128_sb, t128_d)
        f2t_sb = cpool.tile([H, 2], F32, tag="f2t", name="f2t_sb")
        nc.sync.dma_start(f2t_sb, f2t_d)
        if n16:
            ctp16 = cpool.tile([H, 2 * N], BF16, tag="ctp16", name="ctp16")
            nc.sync.dma_start(ctp16, ctp16_d)
            cmt16 = cpool.tile([H, 2 * N], BF16, tag="cmt16", name="cmt16")
            nc.sync.dma_start(cmt16, cmt16_d)
            cm16 = cpool.tile([H, 2 * N], BF16, tag="cm16", name="cm16")
            nc.sync.dma_start(cm16, cm16_d)
            cm416 = cpool.tile([H, 2 * N], BF16, tag="cm416", name="cm416")
            nc.sync.dma_start(cm416, cm416_d)

        # ---- hoisted x DMAs for the first processed bf16 slots ----
        xt = {}
        for b in procorder[:3]:
            if cfg[b] == FULL:
                continue
            xs = wpool.tile([H, CH * W_IMG], BF16, tag="x16", bufs=6, name=f"x_{b}")
            nc.sync.dma_start(xs, x16_d[x16_idx[b]])
            xt[b] = xs

        if n32:
            ctp32 = cpool.tile([H, 2 * N], F32R, tag="ctp32", name="ctp32")
            nc.sync.dma_start(ctp32, ctp32_d)
            cmt32 = cpool.tile([H, 2 * N], F32R, tag="cmt32", name="cmt32")
            nc.sync.dma_start(cmt32, cmt32_d)
            cm32 = cpool.tile([H, 2 * N], F32R, tag="cm32", name="cm32")
            nc.sync.dma_start(cm32, cm32_d)
            cm432 = cpool.tile([H, 2 * N], F32R, tag="cm432", name="cm432")
            nc.sync.dma_start(cm432, cm432_d)

        # ---- PE warmup: junk matmuls with no data deps (HAM ramp) ----
        wdt = BF16 if n16 else F32R
        wtile = cpool.tile([H, H], wdt, tag="warm", name="warm")
        nc.vector.memset(wtile, 0.0)
        pj = ppool.tile([H, H], F32, tag="ps2", name="pjunk")
        for _ in range(10):
            nc.tensor.matmul(pj, wtile, wtile, start=True, stop=True)

        # ---- blur schedule ----
        tbias_sb = cpool.tile([1, 1], F32, tag="tbias", name="tbias_sb")
        nc.vector.memset(tbias_sb, TAU_BIAS)
        tau_sb = cpool.tile([1, n_b], F32, tag="tau", name="tau_sb")
        nc.scalar.activation(tau_sb, t_sb, ACTF.Exp, bias=tbias_sb, scale=TAU_SCALE)
        ntau_sb = cpool.tile([1, n_b], F32, tag="ntau", name="ntau_sb")
        nc.vector.tensor_scalar_mul(ntau_sb, tau_sb, -1.0)
        # per-partition copy for the separable-mask path ([p,1] scale APs)
        tbias128 = cpool.tile([H, 1], F32, tag="tbias128", name="tbias128")
        nc.vector.memset(tbias128, TAU_BIAS)
        hbias128 = cpool.tile([H, 1], F32, tag="hbias128", name="hbias128")
        nc.vector.memset(hbias128, HALF_LN999)
        tau128 = cpool.tile([H, n_b], F32, tag="tau128", name="tau128")
        nc.scalar.activation(tau128, t128_sb, ACTF.Exp, bias=tbias128, scale=TAU_SCALE)
        ntau128 = cpool.tile([H, n_b], F32, tag="ntau128", name="ntau128")
        nc.vector.tensor_scalar_mul(ntau128, tau128, -1.0)

        mask_sb = {}  # FULL slot -> mask tile (0.999*fade, thresholded)

        def build_mask(b):
            u = wpool.tile([1, N], F32R, tag="u32", name=f"u_{b}")
            nc.scalar.activation(
                u, f2_sb, ACTF.Exp, scale=ntau_sb[0:1, b : b + 1]
            )
            psm = ppool.tile([H, 2 * N], F32, tag="ps3", name=f"psm_{b}")
            for m2 in range(2):
                nc.tensor.matmul(
                    psm[:, m2 * N : (m2 + 1) * N],
                    u[0:1, m2 * H : m2 * H + H],
                    u[0:1, 0:N],
                    start=True,
                    stop=True,
                )
            fade = wpool.tile([H, 2 * N], F32, tag="fade", name=f"fade_{b}")
            nc.scalar.activation(fade, psm, ACTF.Copy, scale=1.0 - MIN_SCALE)
            m = wpool.tile([H, 2 * N], F32, tag="mask", bufs=n32 + 2, name=f"mask_{b}")
            nc.vector.scalar_tensor_tensor(
                m, fade, THR, fade, op0=ALU.is_ge, op1=ALU.mult
            )
            mask_sb[b] = m

        for b in full_slots:
            build_mask(b)

        def get_x(b, dt, tensor, idx):
            if b in xt:
                return xt[b]
            xs = wpool.tile(
                [H, CH * W_IMG], dt, tag="x32" if dt is F32R else "x16",
                bufs=2 if dt is F32R else 6, name=f"x_{b}",
            )
            nc.sync.dma_start(xs, tensor[idx])
            return xs

        def emit_bf16(b):
            s = cfg[b]
            kb = 1 if s <= H else 2
            kw = [min(s, H)] if kb == 1 else [H, s - H]
            xs = get_x(b, BF16, x16_d, x16_idx[b])
            # separable mask 0.999*u(x)u folded into per-slot copies of the
            # stage-3 rhs (row factor u_n) and stage-4 weights (col u_k);
            # each u carries sqrt(0.999) via the exp bias; no threshold.
            uT = wpool.tile([H, 2], F32, tag="ut", bufs=4, name=f"ut_{b}")
            nc.scalar.activation(
                uT, f2t_sb, ACTF.Exp, bias=hbias128,
                scale=ntau128[:, b : b + 1],
            )
            load["act"] += 200
            cmu = wpool.tile([H, kb * N], BF16, tag="cmu", bufs=3, name=f"cmu_{b}")
            cm4u = wpool.tile([H, kb * N], BF16, tag="cm4u", bufs=3, name=f"cm4u_{b}")
            for nn in range(kb):
                nc.vector.tensor_scalar_mul(
                    cmu[0 : kw[nn], nn * N : (nn + 1) * N],
                    cm16[0 : kw[nn], nn * N : (nn + 1) * N],
                    uT[0 : kw[nn], nn : nn + 1],
                )
                nc.vector.tensor_scalar_mul(
                    cm4u[0 : kw[nn], nn * N : (nn + 1) * N],
                    cm416[0 : kw[nn], nn * N : (nn + 1) * N],
                    uT[0 : kw[nn], nn : nn + 1],
                )
                load["dve"] += 2 * (0.35 * N + 200)
            s1, s2, s3 = {}, {}, {}
            # stage 1: S1 = (C X)^T -> [w, k<s]; m = w-half, r = row parity
            for j in range(CH):
                p1 = ppool.tile([H, 2 * s], F32, tag="ps1", name=f"p1_{b}_{j}")
                for m in range(2):
                    for r in range(2):
                        nc.tensor.matmul(
                            p1[:, m * s : (m + 1) * s],
                            xs[:, j * W_IMG + r * N + m * H : j * W_IMG + r * N + m * H + H],
                            ctp16[:, r * N : r * N + s],
                            start=(r == 0),
                            stop=(r == 1),
                        )
                t1 = wpool.tile([H, 2 * s], BF16, tag=f"s1x{j}", bufs=3, name=f"s1_{b}_{j}")
                evict(t1, p1, 2 * s, pick(2 * s))
                s1[j] = t1
            # stage 2 + mask: S2T = mask (*) (C X C^T)^T -> [n<s, k<s]
            for j in range(CH):
                p2 = ppool.tile([H, kb * s], F32, tag="ps2", name=f"p2_{b}_{j}")
                for m2 in range(kb):
                    for ww in range(2):
                        nc.tensor.matmul(
                            p2[0 : kw[m2], m2 * s : m2 * s + s],
                            cmt16[:, ww * N + m2 * H : ww * N + m2 * H + kw[m2]],
                            s1[j][:, ww * s : (ww + 1) * s],
                            start=(ww == 0),
                            stop=(ww == 1),
                        )
                t2 = wpool.tile([H, kb * s], BF16, tag=f"s2x{j}", bufs=3, name=f"s2_{b}_{j}")
                eng = pick(kb * s)
                for m2 in range(kb):
                    sl = (slice(0, kw[m2]), slice(m2 * s, m2 * s + s))
                    evict(t2[sl], p2[sl], s, eng)
                s2[j] = t2
            # stage 3: S3 = S2m @ C -> [k<s, h]
            for j in range(CH):
                p3 = ppool.tile([H, kb * N], F32, tag="ps3", name=f"p3_{b}_{j}")
                for m3 in range(kb):
                    for nn in range(kb):
                        nc.tensor.matmul(
                            p3[0 : kw[m3], m3 * N : m3 * N + N],
                            s2[j][0 : kw[nn], nn * s + m3 * H : nn * s + m3 * H + kw[m3]],
                            cmu[0 : kw[nn], nn * N : nn * N + N],
                            start=(nn == 0),
                            stop=(nn == kb - 1),
                        )
                t3 = wpool.tile([H, kb * N], BF16, tag=f"s3x{j}", bufs=3, name=f"s3_{b}_{j}")
                eng = pick(kb * N)
                for m3 in range(kb):
                    sl = (slice(0, kw[m3]), slice(m3 * N, m3 * N + N))
                    evict(t3[sl], p3[sl], N, eng)
                s3[j] = t3
            # stage 4: Z = C^T @ S3 in row-pair layout; out eviction fused
            o = wpool.tile([H, CH * W_IMG], BF16, tag="o", bufs=3, name=f"o_{b}")
            for j in range(CH):
                p4 = ppool.tile([H, 2 * N], F32, tag="ps4", name=f"p4_{b}_{j}")
                for m in range(2):
                    for kp in range(kb):
                        nc.tensor.matmul(
                            p4[:, m * N : (m + 1) * N],
                            cm4u[0 : kw[kp], kp * N + m * H : kp * N + m * H + H],
                            s3[j][0 : kw[kp], kp * N : kp * N + N],
                            start=(kp == 0),
                            stop=(kp == kb - 1),
                        )
                osl = o[:, j * W_IMG : (j + 1) * W_IMG]
                if pick(2 * N) == "dve":
                    nc.vector.scalar_tensor_tensor(
                        osl, xs[:, j * W_IMG : (j + 1) * W_IMG], MIN_SCALE, p4,
                        op0=ALU.mult, op1=ALU.add,
                    )
                else:
                    nc.scalar.copy(osl, p4)  # drops 0.001*X (~1e-3)
            nc.sync.dma_start(y_d[b], o)

        def emit_full(b):
            xs = get_x(b, F32R, x32_d, x32_idx[b])
            s1, s2, s3 = {}, {}, {}
            for j in range(CH):
                p1 = ppool.tile([H, 2 * N], F32, tag="ps1", name=f"p1_{b}_{j}")
                for m in range(2):
                    for r in range(2):
                        nc.tensor.matmul(
                            p1[:, m * N : (m + 1) * N],
                            xs[:, j * W_IMG + r * N + m * H : j * W_IMG + r * N + m * H + H],
                            ctp32[:, r * N : (r + 1) * N],
                            start=(r == 0),
                            stop=(r == 1),
                        )
                t1 = wpool.tile([H, 2 * N], F32R, tag=f"s1d{j}", bufs=3, name=f"s1_{b}_{j}")
                evict(t1, p1, 2 * N, pick(2 * N))
                s1[j] = t1
            for j in range(CH):
                p2 = ppool.tile([H, 2 * N], F32, tag="ps2", name=f"p2_{b}_{j}")
                for m2 in range(2):
                    for ww in range(2):
                        nc.tensor.matmul(
                            p2[:, m2 * N : (m2 + 1) * N],
                            cmt32[:, ww * N + m2 * H : ww * N + m2 * H + H],
                            s1[j][:, ww * N : (ww + 1) * N],
                            start=(ww == 0),
                            stop=(ww == 1),
                        )
                t2 = wpool.tile([H, 2 * N], F32R, tag=f"s2d{j}", bufs=3, name=f"s2_{b}_{j}")
                load["dve"] += 1.10 * 2 * N + 250
                nc.vector.tensor_mul(t2, p2, mask_sb[b])
                s2[j] = t2
            for j in range(CH):
                p3 = ppool.tile([H, 2 * N], F32, tag="ps3", name=f"p3_{b}_{j}")
                for m3 in range(2):
                    for nn in range(2):
                        nc.tensor.matmul(
                            p3[:, m3 * N : (m3 + 1) * N],
                            s2[j][:, nn * N + m3 * H : nn * N + m3 * H + H],
                            cm32[:, nn * N : (nn + 1) * N],
                            start=(nn == 0),
                            stop=(nn == 1),
                        )
                t3 = wpool.tile([H, 2 * N], F32R, tag=f"s3d{j}", bufs=3, name=f"s3_{b}_{j}")
                evict(t3, p3, 2 * N, pick(2 * N))
                s3[j] = t3
            o = wpool.tile([H, CH * W_IMG], BF16, tag="o", bufs=3, name=f"o_{b}")
            for j in range(CH):
                p4 = ppool.tile([H, 2 * N], F32, tag="ps4", name=f"p4_{b}_{j}")
                for m in range(2):
                    for kp in range(2):
                        nc.tensor.matmul(
                            p4[:, m * N : (m + 1) * N],
                            cm432[:, kp * N + m * H : kp * N + m * H + H],
                            s3[j][:, kp * N : (kp + 1) * N],
                            start=(kp == 0),
                            stop=(kp == 1),
                        )
                load["dve"] += 1.10 * 2 * N + 250
                nc.vector.scalar_tensor_tensor(
                    o[:, j * W_IMG : (j + 1) * W_IMG],
                    xs[:, j * W_IMG : (j + 1) * W_IMG],
                    MIN_SCALE,
                    p4,
                    op0=ALU.mult,
                    op1=ALU.add,
                )
            nc.sync.dma_start(y_d[b], o)

        for b in procorder:
            if cfg[b] == FULL:
                emit_full(b)
            else:
                emit_bf16(b)

    nc.compile()
    return nc


def host_constants():
    n = np.arange(N, dtype=np.float64)
    C = np.cos(np.pi * (n[None, :] + 0.5) * n[:, None] / N)
    scale = np.where(n[:, None] == 0, np.sqrt(1.0 / N), np.sqrt(2.0 / N))
    C = (C * scale).astype(np.float32)
    f = (np.pi * np.arange(N) / N).astype(np.float32)
    f2 = (f * f).astype(np.float32)
    A = np.ascontiguousarray(C.T)  # A[h, k] = C[k, h]
    # ctp[p, r*N+k] = C[k, 2p+r]          (stage-1 rhs, rows even/odd split)
    ctp = A.reshape(H, 2, N).reshape(H, 2 * N)
    # cmt[p, ww*N+n] = C[n, ww*128+p]     (stage-2 weights, natural w-halves)
    cmt = A.reshape(2, H, N).transpose(1, 0, 2).reshape(H, 2 * N)
    # cm[p, nn*N+h] = C[nn*128+p, h]      (stage-3 rhs, natural rows)
    cm = C.reshape(2, H, N).transpose(1, 0, 2).reshape(H, 2 * N)
    # cm4[p, kp*N+m*H+w] = C[kp*128+p, 2w+m]  (stage-4 weights, cols split)
    cm4 = C.reshape(2, H, H, 2).transpose(1, 0, 3, 2).reshape(H, 2 * N)
    return (
        np.ascontiguousarray(ctp),
        np.ascontiguousarray(cmt),
        np.ascontiguousarray(cm),
        np.ascontiguousarray(cm4),
        f2,
    )


def s_of_t(t):
    """Per-item axis cutoff: smallest s (with safety margin) such that
    every kept fade entry (>=0.01) has both indices < s."""
    t64 = np.asarray(t, dtype=np.float64)
    sigma = np.exp(np.log(MIN_BLUR) * (1 - t64) + np.log(MAX_BLUR) * t64)
    tau = sigma * sigma / 2.0
    lim = np.log(100.0) / tau  # keep (i,j) with f_i^2+f_j^2 <= lim
    imax = np.floor(N * np.sqrt(lim) / np.pi).astype(np.int64)
    return np.minimum(imax + 3, 2 * N).astype(np.int64)


def interleave(img):
    """[3,256,256] fp32 -> [128, 3*512] row-pair layout."""
    return np.ascontiguousarray(
        img.reshape(CH, H, 2, N).transpose(1, 0, 2, 3).reshape(H, CH * W_IMG)
    )


def deinterleave(arr):
    """[128, 3*512] -> [3,256,256]."""
    return arr.reshape(H, CH, 2, N).transpose(1, 0, 2, 3).reshape(CH, N, N)


_CACHE = {}


def _get_nc(cfg):
    if cfg not in _CACHE:
        _CACHE[cfg] = build_nc(cfg)
    return _CACHE[cfg]


def _run(x, t, trace=False, tmpdir=None):
    x = np.ascontiguousarray(np.asarray(x, dtype=np.float32))
    t = np.asarray(t, dtype=np.float32)
    assert x.shape == (B, CH, N, N) and t.shape == (B,)

    s_item = s_of_t(t)
    # sort densest first, deal round-robin: slot b of core c gets item
    # order[b*8+c]; slot config = group max (sorted -> first of group)
    order = np.argsort(-s_item, kind="stable")
    cfg = []
    for b in range(BPC):
        smax = int(s_item[order[b * NCORES]])
        cfg.append(FULL if smax > N else int(min(N, ((smax + 31) // 32) * 32)))
    cfg = tuple(cfg)
    nc = _get_nc(cfg)

    ctp, cmt, cm, cm4, f2 = host_constants()
    n32 = sum(1 for s in cfg if s == FULL)
    in_maps = []
    for c in range(NCORES):
        items = order[np.arange(BPC) * NCORES + c]  # slot b -> batch index
        x32 = np.empty((n32, H, CH * W_IMG), np.float32) if n32 else None
        x16 = (
            np.empty((BPC - n32, H, CH * W_IMG), BF16_NP) if n32 < BPC else None
        )
        for b in range(BPC):
            il = interleave(x[items[b]])
            if cfg[b] == FULL:
                x32[b] = il
            else:
                x16[b - n32] = il.astype(BF16_NP)
        tc_ = np.ascontiguousarray(t[items].reshape(1, BPC))
        f = (np.pi * np.arange(N) / N).astype(np.float32)
        f2t = np.ascontiguousarray((f * f).reshape(2, H).T)
        m = {
            "t": tc_,
            "f2": f2.reshape(1, N),
            "t128": np.ascontiguousarray(np.broadcast_to(tc_, (H, BPC))),
            "f2t": f2t,
        }
        if n32:
            m["x32"] = x32
            m["ctp32"], m["cmt32"], m["cm32"], m["cm432"] = ctp, cmt, cm, cm4
        if n32 < BPC:
            m["x16"] = x16
            m["ctp16"] = ctp.astype(BF16_NP)
            m["cmt16"] = cmt.astype(BF16_NP)
            m["cm16"] = cm.astype(BF16_NP)
            m["cm416"] = cm4.astype(BF16_NP)
        in_maps.append(m)
    res = run_bass_kernel_spmd(
        nc, in_maps, core_ids=list(range(NCORES)), trace=trace, tmpdir=tmpdir
    )
    out = np.empty_like(x)
    for c in range(NCORES):
        items = order[np.arange(BPC) * NCORES + c]
        y = res.results[c]["y"].astype(np.float32)
        for b in range(BPC):
            out[items[b]] = deinterleave(y[b])
    return out, res


def kernel(x, t):
    out, _ = _run(x, t)
    return out


def kernel_with_profile(x, t, tmpdir=None):
    out, res = _run(x, t, trace=True, tmpdir=tmpdir)
    return out, res


# revision 25
# speedup vs baseline: 1.1537x; 1.1016x over previous
"""DCT-blur kernel for 8 Trainium2 NeuronCores.

Computes, per image X [256,256]:
    out = C^T @ (M_b (*) (C @ X @ C^T)) @ C
where C is the orthonormal DCT-II matrix and M_b a per-batch-item
frequency fade mask derived from t[b]:
    sigma = exp(log(.5)(1-t) + log(20)t); tau = sigma^2/2
    fade[i,j] = exp(-(f_i^2+f_j^2) tau);  fade<0.01 -> 0
    M = fade*(1-0.001) + 0.001,   f_i = pi*i/256
Since C is orthonormal the 0.001 floor is pulled out exactly:
    out = 0.001*X + 0.999 * C^T @ (fade_clamped (*) (C X C^T)) @ C

The clamped fade has quarter-disk support with axis cutoff s(t) =
O(40^-t) indices.  Per batch item the cutoff (rounded up to 32) is
baked into the program:
  * s <= 256 (~96% of items): bf16 path whose four matmul stages touch
    only the first s frequencies, in 128-blocks (kb = ceil(s/128)).
    bf16 runs any free dim at 1 cycle/row (fp32r needs >=256) and its
    128-wide weights get fast-weight-load, so LDWEIGHTS hides.
  * s > 256: full 16-matmul fp32r path (minimal blur; these items
    dominate the output absmax so they keep the precise dtype).

Numerics vs the 2e-2 gate: bf16 output rounding ~2e-3; bf16 operand
rounding on partially-blurred items ~3e-3 (mask-filtered for small s);
for s<=128 the fade<0.01 threshold is skipped (each borderline
coefficient spreads as a cos wave of amplitude <=0.01*|Y|*2/256, total
~1e-4); slots whose output is evicted on the Scalar engine drop the
0.001*X term (~1e-3).  Measured total ~3-5e-3.

Layout: images live per-partition as row pairs (partition p holds rows
2p, 2p+1), so every x / y DMA is a straight [128, 3*512] copy (2KB
lines, one dma_start per 3-channel slot).  The DCT constants absorb the
permutation: stage-1 rhs ctp is C^T row-split even/odd, stage-4 weights
cm4 are C column-split even/odd, making stage-4 PSUM output land
exactly in row-pair layout, elementwise-aligned with the x tile for the
final + 0.001*X.

Engine budget: every PSUM eviction runs at 1 elem/cycle/lane (fp32
PSUM source), so the ~70k eviction columns are split between ACT and
DVE by a greedy ns-estimate balancer; the mask multiply doubles as the
stage-2 eviction and the 0.001*X add as stage-4's (DVE only).  Heavy
(full/medium) and light slots are interleaved so the PE never idles
long enough to re-throttle (HAM) and eviction load stays even; the
first and last slots are the sparsest.  A few junk matmuls plus
prebuilt masks warm the PE during the input-DMA head.

Sharding: pure data parallel, batch 128 -> 16 slots per core.  The
host sorts items by cutoff (descending), deals round-robin, and each
slot's config is the max over its 8 cores' items, so one SPMD program
serves all cores.
"""

from contextlib import ExitStack

import numpy as np
import ml_dtypes

import concourse.bass as bass
import concourse.tile as tile
from concourse import bacc, mybir
from concourse.bass_utils import run_bass_kernel_spmd

B, CH, N = 128, 3, 256
NCORES = 8
BPC = B // NCORES  # batch items (slots) per core
H = N // 2  # 128 = partition count
W_IMG = 2 * N  # 512 floats per partition per image (row pair)

MIN_BLUR, MAX_BLUR, MIN_SCALE = 0.5, 20.0, 0.001

F32 = mybir.dt.float32
F32R = mybir.dt.float32r
BF16 = mybir.dt.bfloat16
BF16_NP = ml_dtypes.bfloat16
ALU = mybir.AluOpType
ACTF = mybir.ActivationFunctionType

FULL = 1000  # cfg value marking the fp32r full path (s > 256)

TAU_SCALE = float(2.0 * np.log(MAX_BLUR / MIN_BLUR))
TAU_BIAS = float(np.log(0.5 * MIN_BLUR * MIN_BLUR))
THR = float(np.float32(np.float32(1.0 - MIN_SCALE) * np.float32(0.01)))
HALF_LN999 = float(0.5 * np.log(1.0 - MIN_SCALE))


def _proc_order(cfg):
    """Interleave heavy (full/medium) and light slots; start and end light.
    fp32r (FULL) slots go mid-sequence so their fp32 constants (DMA'd after
    the bf16 ones) have landed."""
    n_b = len(cfg)
    full = [b for b in range(n_b) if cfg[b] == FULL]
    med = [b for b in range(n_b) if 128 < cfg[b] < FULL]
    heavy = med[:2] + full + med[2:]
    light = [b for b in range(n_b) if cfg[b] <= 128][::-1]  # sparsest first
    if not light:
        return heavy
    order = [light[0]]
    li = 1
    for h in heavy:
        order.append(h)
        if li < len(light):
            order.append(light[li])
            li += 1
    order += light[li:]
    return order


def build_nc(cfg):
    """cfg: per-slot axis cutoff in dealt order; FULL = fp32r path, else
    s in {32,64,...,256} = bf16 block path."""
    n_b = len(cfg)
    full_slots = [b for b in range(n_b) if cfg[b] == FULL]
    bf_slots = [b for b in range(n_b) if cfg[b] != FULL]
    n32, n16 = len(full_slots), len(bf_slots)
    x32_idx = {b: i for i, b in enumerate(full_slots)}
    x16_idx = {b: i for i, b in enumerate(bf_slots)}
    procorder = _proc_order(cfg)

    nc = bacc.Bacc(
        "TRN2",
        target_bir_lowering=False,
        debug=False,
        num_devices=NCORES,
    )
    if n32:
        x32_d = nc.dram_tensor("x32", [n32, H, CH * W_IMG], F32R, kind="ExternalInput").ap()
        ctp32_d = nc.dram_tensor("ctp32", [H, 2 * N], F32R, kind="ExternalInput").ap()
        cmt32_d = nc.dram_tensor("cmt32", [H, 2 * N], F32R, kind="ExternalInput").ap()
        cm32_d = nc.dram_tensor("cm32", [H, 2 * N], F32R, kind="ExternalInput").ap()
        cm432_d = nc.dram_tensor("cm432", [H, 2 * N], F32R, kind="ExternalInput").ap()
    if n16:
        x16_d = nc.dram_tensor("x16", [n16, H, CH * W_IMG], BF16, kind="ExternalInput").ap()
        ctp16_d = nc.dram_tensor("ctp16", [H, 2 * N], BF16, kind="ExternalInput").ap()
        cmt16_d = nc.dram_tensor("cmt16", [H, 2 * N], BF16, kind="ExternalInput").ap()
        cm16_d = nc.dram_tensor("cm16", [H, 2 * N], BF16, kind="ExternalInput").ap()
        cm416_d = nc.dram_tensor("cm416", [H, 2 * N], BF16, kind="ExternalInput").ap()
    t_d = nc.dram_tensor("t", [1, n_b], F32, kind="ExternalInput").ap()
    f2_d = nc.dram_tensor("f2", [1, N], F32, kind="ExternalInput").ap()
    t128_d = nc.dram_tensor("t128", [H, n_b], F32, kind="ExternalInput").ap()
    f2t_d = nc.dram_tensor("f2t", [H, 2], F32, kind="ExternalInput").ap()
    y_d = nc.dram_tensor("y", [n_b, H, CH * W_IMG], BF16, kind="ExternalOutput").ap()

    # greedy ACT/DVE eviction balancer (ns estimates incl. fixed overhead)
    load = {"act": 0.0, "dve": 0.0}

    def pick(cols):
        a = load["act"] + 0.85 * cols + 250
        v = load["dve"] + 1.10 * cols + 250
        if a <= v:
            load["act"] = a
            return "act"
        load["dve"] = v
        return "dve"

    with tile.TileContext(nc) as tc, ExitStack() as ctx:
        cpool = ctx.enter_context(tc.tile_pool(name="consts", bufs=1))
        wpool = ctx.enter_context(tc.tile_pool(name="work", bufs=2))
        ppool = ctx.enter_context(tc.tile_pool(name="psum", bufs=2, space="PSUM"))

        def evict(dst, src, cols, eng):
            if eng == "act":
                nc.scalar.copy(dst, src)
            else:
                nc.vector.tensor_copy(dst, src)

        # ---- small constants first ----
        t_sb = cpool.tile([1, n_b], F32, tag="t", name="t_sb")
        nc.sync.dma_start(t_sb, t_d)
        f2_sb = cpool.tile([1, N], F32, tag="f2", name="f2_sb")
        nc.sync.dma_start(f2_sb, f2_d)
        t128_sb = cpool.tile([H, n_b], F32, tag="t128", name="t128_sb")
        nc.sync.dma_start(t128_sb, t128_d)
        f2t_sb = cpool.tile([H, 2], F32, tag="f2t", name="f2t_sb")
        nc.sync.dma_start(f2t_sb, f2t_d)
        if n16:
            ctp16 = cpool.tile([H, 2 * N], BF16, tag="ctp16", name="ctp16")
            nc.sync.dma_start(ctp16, ctp16_d)
            cmt16 = cpool.tile([H, 2 * N], BF16, tag="cmt16", name="cmt16")
            nc.sync.dma_start(cmt16, cmt16_d)
            cm16 = cpool.tile([H, 2 * N], BF16, tag="cm16", name="cm16")
            nc.sync.dma_start(cm16, cm16_d)
            cm416 = cpool.tile([H, 2 * N], BF16, tag="cm416", name="cm416")
            nc.sync.dma_start(cm416, cm416_d)

        # ---- hoisted x DMAs for the first processed bf16 slots ----
        xt = {}
        for b in procorder[:3]:
            if cfg[b] == FULL:
                continue
            xs = wpool.tile([H, CH * W_IMG], BF16, tag="x16", bufs=6, name=f"x_{b}")
            nc.sync.dma_start(xs, x16_d[x16_idx[b]])
            xt[b] = xs

        if n32:
            ctp32 = cpool.tile([H, 2 * N], F32R, tag="ctp32", name="ctp32")
            nc.sync.dma_start(ctp32, ctp32_d)
            cmt32 = cpool.tile([H, 2 * N], F32R, tag="cmt32", name="cmt32")
            nc.sync.dma_start(cmt32, cmt32_d)
            cm32 = cpool.tile([H, 2 * N], F32R, tag="cm32", name="cm32")
            nc.sync.dma_start(cm32, cm32_d)
            cm432 = cpool.tile([H, 2 * N], F32R, tag="cm432", name="cm432")
            nc.sync.dma_start(cm432, cm432_d)

        # ---- PE warmup: junk matmuls with no data deps (HAM ramp) ----
        wdt = BF16 if n16 else F32R
        wtile = cpool.tile([H, H], wdt, tag="warm", name="warm")
        nc.vector.memset(wtile, 0.0)
        pj = ppool.tile([H, H], F32, tag="ps2", name="pjunk")
        for _ in range(10):
            nc.tensor.matmul(pj, wtile, wtile, start=True, stop=True)

        # ---- blur schedule ----
        tbias_sb = cpool.tile([1, 1], F32, tag="tbias", name="tbias_sb")
        nc.vector.memset(tbias_sb, TAU_BIAS)
        tau_sb = cpool.tile([1, n_b], F32, tag="tau", name="tau_sb")
        nc.scalar.activation(tau_sb, t_sb, ACTF.Exp, bias=tbias_sb, scale=TAU_SCALE)
        ntau_sb = cpool.tile([1, n_b], F32, tag="ntau", name="ntau_sb")
        nc.vector.tensor_scalar_mul(ntau_sb, tau_sb, -1.0)
        # per-partition copy for the separable-mask path ([p,1] scale APs)
        tbias128 = cpool.tile([H, 1], F32, tag="tbias128", name="tbias128")
        nc.vector.memset(tbias128, TAU_BIAS)
        hbias128 = cpool.tile([H, 1], F32, tag="hbias128", name="hbias128")
        nc.vector.memset(hbias128, HALF_LN999)
        tau128 = cpool.tile([H, n_b], F32, tag="tau128", name="tau128")
        nc.scalar.activation(tau128, t128_sb, ACTF.Exp, bias=tbias128, scale=TAU_SCALE)
        ntau128 = cpool.tile([H, n_b], F32, tag="ntau128", name="ntau128")
        nc.vector.tensor_scalar_mul(ntau128, tau128, -1.0)

        mask_sb = {}  # FULL slot -> mask tile (0.999*fade, thresholded)

        def build_mask(b):
            u = wpool.tile([1, N], F32R, tag="u32", name=f"u_{b}")
            nc.scalar.activation(
                u, f2_sb, ACTF.Exp, scale=ntau_sb[0:1, b : b + 1]
            )
            psm = ppool.tile([H, 2 * N], F32, tag="ps3", name=f"psm_{b}")
            for m2 in range(2):
                nc.tensor.matmul(
                    psm[:, m2 * N : (m2 + 1) * N],
                    u[0:1, m2 * H : m2 * H + H],
                    u[0:1, 0:N],
                    start=True,
                    stop=True,
                )
            fade = wpool.tile([H, 2 * N], F32, tag="fade", name=f"fade_{b}")
            nc.scalar.activation(fade, psm, ACTF.Copy, scale=1.0 - MIN_SCALE)
            m = wpool.tile([H, 2 * N], F32, tag="mask", bufs=n32 + 2, name=f"mask_{b}")
            nc.vector.scalar_tensor_tensor(
                m, fade, THR, fade, op0=ALU.is_ge, op1=ALU.mult
            )
            mask_sb[b] = m

        for b in full_slots:
            build_mask(b)

        def get_x(b, dt, tensor, idx):
            if b in xt:
                return xt[b]
            xs = wpool.tile(
                [H, CH * W_IMG], dt, tag="x32" if dt is F32R else "x16",
                bufs=2 if dt is F32R else 6, name=f"x_{b}",
            )
            nc.sync.dma_start(xs, tensor[idx])
            return xs

        def emit_bf16_s12(b):
            s = cfg[b]
            kb = 1 if s <= H else 2
            kw = [min(s, H)] if kb == 1 else [H, s - H]
            xs = get_x(b, BF16, x16_d, x16_idx[b])
            # separable mask 0.999*u(x)u folded into per-slot copies of the
            # stage-3 rhs (row factor u_n) and stage-4 weights (col u_k);
            # each u carries sqrt(0.999) via the exp bias; no threshold.
            uT = wpool.tile([H, 2], F32, tag="ut", bufs=4, name=f"ut_{b}")
            nc.scalar.activation(
                uT, f2t_sb, ACTF.Exp, bias=hbias128,
                scale=ntau128[:, b : b + 1],
            )
            load["act"] += 200
            cmu = wpool.tile([H, kb * N], BF16, tag="cmu", bufs=3, name=f"cmu_{b}")
            cm4u = wpool.tile([H, kb * N], BF16, tag="cm4u", bufs=3, name=f"cm4u_{b}")
            for nn in range(kb):
                nc.vector.tensor_scalar_mul(
                    cmu[0 : kw[nn], nn * N : (nn + 1) * N],
                    cm16[0 : kw[nn], nn * N : (nn + 1) * N],
                    uT[0 : kw[nn], nn : nn + 1],
                )
                nc.vector.tensor_scalar_mul(
                    cm4u[0 : kw[nn], nn * N : (nn + 1) * N],
                    cm416[0 : kw[nn], nn * N : (nn + 1) * N],
                    uT[0 : kw[nn], nn : nn + 1],
                )
                load["dve"] += 2 * (0.35 * N + 200)
            s1, s2, s3 = {}, {}, {}
            # stage 1: S1 = (C X)^T -> [w, k<s]; m = w-half, r = row parity
            for j in range(CH):
                p1 = ppool.tile([H, 2 * s], F32, tag="ps1", name=f"p1_{b}_{j}")
                for m in range(2):
                    for r in range(2):
                        nc.tensor.matmul(
                            p1[:, m * s : (m + 1) * s],
                            xs[:, j * W_IMG + r * N + m * H : j * W_IMG + r * N + m * H + H],
                            ctp16[:, r * N : r * N + s],
                            start=(r == 0),
                            stop=(r == 1),
                        )
                t1 = wpool.tile([H, 2 * s], BF16, tag=f"s1x{j}", bufs=3, name=f"s1_{b}_{j}")
                evict(t1, p1, 2 * s, pick(2 * s))
                s1[j] = t1
            # stage 2 + mask: S2T = mask (*) (C X C^T)^T -> [n<s, k<s]
            for j in range(CH):
                p2 = ppool.tile([H, kb * s], F32, tag="ps2", name=f"p2_{b}_{j}")
                for m2 in range(kb):
                    for ww in range(2):
                        nc.tensor.matmul(
                            p2[0 : kw[m2], m2 * s : m2 * s + s],
                            cmt16[:, ww * N + m2 * H : ww * N + m2 * H + kw[m2]],
                            s1[j][:, ww * s : (ww + 1) * s],
                            start=(ww == 0),
                            stop=(ww == 1),
                        )
                t2 = wpool.tile([H, kb * s], BF16, tag=f"s2x{j}", bufs=3, name=f"s2_{b}_{j}")
                eng = pick(kb * s)
                for m2 in range(kb):
                    sl = (slice(0, kw[m2]), slice(m2 * s, m2 * s + s))
                    evict(t2[sl], p2[sl], s, eng)
                s2[j] = t2
            return dict(s=s, kb=kb, kw=kw, xs=xs, s2=s2, cmu=cmu, cm4u=cm4u)

        def emit_bf16_s34(b, st):
            s, kb, kw, xs, s2 = st["s"], st["kb"], st["kw"], st["xs"], st["s2"]
            cmu, cm4u = st["cmu"], st["cm4u"]
            s3 = {}
            # stage 3: S3 = S2m @ C -> [k<s, h]
            for j in range(CH):
                p3 = ppool.tile([H, kb * N], F32, tag="ps3", name=f"p3_{b}_{j}")
                for m3 in range(kb):
                    for nn in range(kb):
                        nc.tensor.matmul(
                            p3[0 : kw[m3], m3 * N : m3 * N + N],
                            s2[j][0 : kw[nn], nn * s + m3 * H : nn * s + m3 * H + kw[m3]],
                            cmu[0 : kw[nn], nn * N : nn * N + N],
                            start=(nn == 0),
                            stop=(nn == kb - 1),
                        )
                t3 = wpool.tile([H, kb * N], BF16, tag=f"s3x{j}", bufs=3, name=f"s3_{b}_{j}")
                eng = pick(kb * N)
                for m3 in range(kb):
                    sl = (slice(0, kw[m3]), slice(m3 * N, m3 * N + N))
                    evict(t3[sl], p3[sl], N, eng)
                s3[j] = t3
            # stage 4: Z = C^T @ S3 in row-pair layout; out eviction fused
            o = wpool.tile([H, CH * W_IMG], BF16, tag="o", bufs=3, name=f"o_{b}")
            for j in range(CH):
                p4 = ppool.tile([H, 2 * N], F32, tag="ps4", name=f"p4_{b}_{j}")
                for m in range(2):
                    for kp in range(kb):
                        nc.tensor.matmul(
                            p4[:, m * N : (m + 1) * N],
                            cm4u[0 : kw[kp], kp * N + m * H : kp * N + m * H + H],
                            s3[j][0 : kw[kp], kp * N : kp * N + N],
                            start=(kp == 0),
                            stop=(kp == kb - 1),
                        )
                osl = o[:, j * W_IMG : (j + 1) * W_IMG]
                if pick(2 * N) == "dve":
                    nc.vector.scalar_tensor_tensor(
                        osl, xs[:, j * W_IMG : (j + 1) * W_IMG], MIN_SCALE, p4,
                        op0=ALU.mult, op1=ALU.add,
                    )
                else:
                    nc.scalar.copy(osl, p4)  # drops 0.001*X (~1e-3)
            nc.sync.dma_start(y_d[b], o)

        def emit_full_s12(b):
            xs = get_x(b, F32R, x32_d, x32_idx[b])
            s1, s2 = {}, {}
            for j in range(CH):
                p1 = ppool.tile([H, 2 * N], F32, tag="ps1", name=f"p1_{b}_{j}")
                for m in range(2):
                    for r in range(2):
                        nc.tensor.matmul(
                            p1[:, m * N : (m + 1) * N],
                            xs[:, j * W_IMG + r * N + m * H : j * W_IMG + r * N + m * H + H],
                            ctp32[:, r * N : (r + 1) * N],
                            start=(r == 0),
                            stop=(r == 1),
                        )
                t1 = wpool.tile([H, 2 * N], F32R, tag=f"s1d{j}", bufs=3, name=f"s1_{b}_{j}")
                evict(t1, p1, 2 * N, pick(2 * N))
                s1[j] = t1
            for j in range(CH):
                p2 = ppool.tile([H, 2 * N], F32, tag="ps2", name=f"p2_{b}_{j}")
                for m2 in range(2):
                    for ww in range(2):
                        nc.tensor.matmul(
                            p2[:, m2 * N : (m2 + 1) * N],
                            cmt32[:, ww * N + m2 * H : ww * N + m2 * H + H],
                            s1[j][:, ww * N : (ww + 1) * N],
                            start=(ww == 0),
                            stop=(ww == 1),
                        )
                t2 = wpool.tile([H, 2 * N], F32R, tag=f"s2d{j}", bufs=3, name=f"s2_{b}_{j}")
                load["dve"] += 1.10 * 2 * N + 250
                nc.vector.tensor_mul(t2, p2, mask_sb[b])
                s2[j] = t2
            return dict(xs=xs, s2=s2)

        def emit_full_s34(b, st):
            xs, s2 = st["xs"], st["s2"]
            s3 = {}
            for j in range(CH):
                p3 = ppool.tile([H, 2 * N], F32, tag="ps3", name=f"p3_{b}_{j}")
                for m3 in range(2):
                    for nn in range(2):
                        nc.tensor.matmul(
                            p3[:, m3 * N : (m3 + 1) * N],
                            s2[j][:, nn * N + m3 * H : nn * N + m3 * H + H],
                            cm32[:, nn * N : (nn + 1) * N],
                            start=(nn == 0),
                            stop=(nn == 1),
                        )
                t3 = wpool.tile([H, 2 * N], F32R, tag=f"s3d{j}", bufs=3, name=f"s3_{b}_{j}")
                evict(t3, p3, 2 * N, pick(2 * N))
                s3[j] = t3
            o = wpool.tile([H, CH * W_IMG], BF16, tag="o", bufs=3, name=f"o_{b}")
            for j in range(CH):
                p4 = ppool.tile([H, 2 * N], F32, tag="ps4", name=f"p4_{b}_{j}")
                for m in range(2):
                    for kp in range(2):
                        nc.tensor.matmul(
                            p4[:, m * N : (m + 1) * N],
                            cm432[:, kp * N + m * H : kp * N + m * H + H],
                            s3[j][:, kp * N : (kp + 1) * N],
                            start=(kp == 0),
                            stop=(kp == 1),
                        )
                load["dve"] += 1.10 * 2 * N + 250
                nc.vector.scalar_tensor_tensor(
                    o[:, j * W_IMG : (j + 1) * W_IMG],
                    xs[:, j * W_IMG : (j + 1) * W_IMG],
                    MIN_SCALE,
                    p4,
                    op0=ALU.mult,
                    op1=ALU.add,
                )
            nc.sync.dma_start(y_d[b], o)

        # 1-slot-lag software pipeline: stages 1-2 of slot i+1 are emitted
        # (and thus scheduled) ahead of stages 3-4 of slot i, so every
        # PSUM-eviction latency hides behind a group of independent matmuls
        prev = None
        for b in procorder:
            st = emit_full_s12(b) if cfg[b] == FULL else emit_bf16_s12(b)
            if prev is not None:
                pb, pst = prev
                if cfg[pb] == FULL:
                    emit_full_s34(pb, pst)
                else:
                    emit_bf16_s34(pb, pst)
            prev = (b, st)
        pb, pst = prev
        if cfg[pb] == FULL:
            emit_full_s34(pb, pst)
        else:
            emit_bf16_s34(pb, pst)

    nc.compile()
    return nc


def host_constants():
    n = np.arange(N, dtype=np.float64)
    C = np.cos(np.pi * (n[None, :] + 0.5) * n[:, None] / N)
    scale = np.where(n[:, None] == 0, np.sqrt(1.0 / N), np.sqrt(2.0 / N))
    C = (C * scale).astype(np.float32)
    f = (np.pi * np.arange(N) / N).astype(np.float32)
    f2 = (f * f).astype(np.float32)
    A = np.ascontiguousarray(C.T)  # A[h, k] = C[k, h]
    # ctp[p, r*N+k] = C[k, 2p+r]          (stage-1 rhs, rows even/odd split)
    ctp = A.reshape(H, 2, N).reshape(H, 2 * N)
    # cmt[p, ww*N+n] = C[n, ww*128+p]     (stage-2 weights, natural w-halves)
    cmt = A.reshape(2, H, N).transpose(1, 0, 2).reshape(H, 2 * N)
    # cm[p, nn*N+h] = C[nn*128+p, h]      (stage-3 rhs, natural rows)
    cm = C.reshape(2, H, N).transpose(1, 0, 2).reshape(H, 2 * N)
    # cm4[p, kp*N+m*H+w] = C[kp*128+p, 2w+m]  (stage-4 weights, cols split)
    cm4 = C.reshape(2, H, H, 2).transpose(1, 0, 3, 2).reshape(H, 2 * N)
    return (
        np.ascontiguousarray(ctp),
        np.ascontiguousarray(cmt),
        np.ascontiguousarray(cm),
        np.ascontiguousarray(cm4),
        f2,
    )


def s_of_t(t):
    """Per-item axis cutoff: smallest s (with safety margin) such that
    every kept fade entry (>=0.01) has both indices < s."""
    t64 = np.asarray(t, dtype=np.float64)
    sigma = np.exp(np.log(MIN_BLUR) * (1 - t64) + np.log(MAX_BLUR) * t64)
    tau = sigma * sigma / 2.0
    lim = np.log(100.0) / tau  # keep (i,j) with f_i^2+f_j^2 <= lim
    imax = np.floor(N * np.sqrt(lim) / np.pi).astype(np.int64)
    return np.minimum(imax + 3, 2 * N).astype(np.int64)


def interleave(img):
    """[3,256,256] fp32 -> [128, 3*512] row-pair layout."""
    return np.ascontiguousarray(
        img.reshape(CH, H, 2, N).transpose(1, 0, 2, 3).reshape(H, CH * W_IMG)
    )


def deinterleave(arr):
    """[128, 3*512] -> [3,256,256]."""
    return arr.reshape(H, CH, 2, N).transpose(1, 0, 2, 3).reshape(CH, N, N)


_CACHE = {}


def _get_nc(cfg):
    if cfg not in _CACHE:
        _CACHE[cfg] = build_nc(cfg)
    return _CACHE[cfg]


def _run(x, t, trace=False, tmpdir=None):
    x = np.ascontiguousarray(np.asarray(x, dtype=np.float32))
    t = np.asarray(t, dtype=np.float32)
    assert x.shape == (B, CH, N, N) and t.shape == (B,)

    s_item = s_of_t(t)
    # sort densest first, deal round-robin: slot b of core c gets item
    # order[b*8+c]; slot config = group max (sorted -> first of group)
    order = np.argsort(-s_item, kind="stable")
    cfg = []
    for b in range(BPC):
        smax = int(s_item[order[b * NCORES]])
        cfg.append(FULL if smax > N else int(min(N, ((smax + 31) // 32) * 32)))
    cfg = tuple(cfg)
    nc = _get_nc(cfg)

    ctp, cmt, cm, cm4, f2 = host_constants()
    n32 = sum(1 for s in cfg if s == FULL)
    in_maps = []
    for c in range(NCORES):
        items = order[np.arange(BPC) * NCORES + c]  # slot b -> batch index
        x32 = np.empty((n32, H, CH * W_IMG), np.float32) if n32 else None
        x16 = (
            np.empty((BPC - n32, H, CH * W_IMG), BF16_NP) if n32 < BPC else None
        )
        for b in range(BPC):
            il = interleave(x[items[b]])
            if cfg[b] == FULL:
                x32[b] = il
            else:
                x16[b - n32] = il.astype(BF16_NP)
        tc_ = np.ascontiguousarray(t[items].reshape(1, BPC))
        f = (np.pi * np.arange(N) / N).astype(np.float32)
        f2t = np.ascontiguousarray((f * f).reshape(2, H).T)
        m = {
            "t": tc_,
            "f2": f2.reshape(1, N),
            "t128": np.ascontiguousarray(np.broadcast_to(tc_, (H, BPC))),
            "f2t": f2t,
        }
        if n32:
            m["x32"] = x32
            m["ctp32"], m["cmt32"], m["cm32"], m["cm432"] = ctp, cmt, cm, cm4
        if n32 < BPC:
            m["x16"] = x16
            m["ctp16"] = ctp.astype(BF16_NP)
            m["cmt16"] = cmt.astype(BF16_NP)
            m["cm16"] = cm.astype(BF16_NP)
            m["cm416"] = cm4.astype(BF16_NP)
        in_maps.append(m)
    res = run_bass_kernel_spmd(
        nc, in_maps, core_ids=list(range(NCORES)), trace=trace, tmpdir=tmpdir
    )
    out = np.empty_like(x)
    for c in range(NCORES):
        items = order[np.arange(BPC) * NCORES + c]
        y = res.results[c]["y"].astype(np.float32)
        for b in range(BPC):
            out[items[b]] = deinterleave(y[b])
    return out, res


def kernel(x, t):
    out, _ = _run(x, t)
    return out


def kernel_with_profile(x, t, tmpdir=None):
    out, res = _run(x, t, trace=True, tmpdir=tmpdir)
    return out, res


# revision 26
# speedup vs baseline: 1.3260x; 1.1493x over previous
"""DCT-blur kernel for 8 Trainium2 NeuronCores.

Computes, per image X [256,256]:
    out = C^T @ (M_b (*) (C @ X @ C^T)) @ C
where C is the orthonormal DCT-II matrix and M_b a per-batch-item
frequency fade mask derived from t[b]:
    sigma = exp(log(.5)(1-t) + log(20)t); tau = sigma^2/2
    fade[i,j] = exp(-(f_i^2+f_j^2) tau);  fade<0.01 -> 0
    M = fade*(1-0.001) + 0.001,   f_i = pi*i/256
Since C is orthonormal the 0.001 floor is pulled out exactly:
    out = 0.001*X + 0.999 * C^T @ (fade_clamped (*) (C X C^T)) @ C

The clamped fade has quarter-disk support with axis cutoff s(t) =
O(40^-t) indices; per batch item the cutoff (rounded up to 32, capped
at 256) is baked into the program, so the four matmul stages touch only
the first s frequencies, in 128-blocks (kb = ceil(s/128)).  Everything
runs in bf16: 1 cycle/row at any free dim (fp32r needs >=256), and the
128-wide weights get fast-weight-load so LDWEIGHTS hides.

The mask is applied WITHOUT the fade<0.01 threshold, making it
separable: 0.999*fade = u (x) u with u_i = exp(-f_i^2 tau + ln(.999)/2).
It is folded into per-slot scaled copies of the stage-3 rhs (row factor
u_n) and stage-4 weights (column factor u_k) — no mask tile, no mask
multiply, nothing mask-related on the PSUM eviction path.  Skipping the
threshold keeps coefficients the reference zeroes, but each such
coefficient is below 0.01*|Y| and spreads as a cos wave of amplitude
2/256, totalling ~1e-3.  Other error sources vs the 2e-2 gate: bf16
operand rounding ~3-5e-3 (mask-filtered for small s), bf16 output
~2e-3, and slots whose output is evicted on the Scalar engine drop the
0.001*X term (~1e-3).

Layout: images live per-partition as row pairs (partition p holds rows
2p, 2p+1), so every x / y DMA is a straight [128, 3*512] copy (2KB
lines, one dma_start per 3-channel slot).  The DCT constants absorb the
permutation: stage-1 rhs ctp is C^T row-split even/odd, stage-4 weights
cm4 are C column-split even/odd, so stage-4 PSUM output lands exactly
in row-pair layout, elementwise-aligned with the x tile.

Matmul chain per image (matmul(out,lhsT,rhs) = lhsT.T@rhs, contracting
the partition dim):
    S1  = (C X)^T      lhsT=X chunks  rhs=ctp      [w, k<s]   4 MM, N=s
    S2T = (C X C^T)^T  lhsT=cmt       rhs=S1       [n<s, k<s] 2kb MM, N=s
    S3  = S2 @ (u*C)   lhsT=S2        rhs=cmu      [k<s, h]   kb^2 MM, N=256
    Z   = (u*C)^T @ S3 lhsT=cm4u      rhs=S3       [128, 512] 2kb MM, N=256
    out = Z (+ 0.001*X on DVE)        during PSUM eviction

Scheduling: every PSUM eviction runs at 1 elem/cycle/lane, split
between ACT and DVE by a greedy ns balancer.  Slots are emitted as a
1-slot-lag software pipeline (stages 1-2 of slot i+1 ahead of stages
3-4 of slot i) so eviction latencies hide behind independent matmuls
and ps1/ps2 banks run concurrently with ps3/ps4.  Heavy and light
slots are interleaved to keep PE duty high (HAM) and eviction load
even; first/last slots are the sparsest; junk matmuls warm the PE
during the input-DMA head.

Sharding: pure data parallel, batch 128 -> 16 slots per core.  The
host sorts items by cutoff (descending), deals round-robin, and each
slot's config is the max over its 8 cores' items, so one SPMD program
serves all cores.
"""

from contextlib import ExitStack

import numpy as np
import ml_dtypes

import concourse.bass as bass
import concourse.tile as tile
from concourse import bacc, mybir
from concourse.bass_utils import run_bass_kernel_spmd

B, CH, N = 128, 3, 256
NCORES = 8
BPC = B // NCORES  # batch items (slots) per core
H = N // 2  # 128 = partition count
W_IMG = 2 * N  # 512 floats per partition per image (row pair)

MIN_BLUR, MAX_BLUR, MIN_SCALE = 0.5, 20.0, 0.001

F32 = mybir.dt.float32
BF16 = mybir.dt.bfloat16
BF16_NP = ml_dtypes.bfloat16
ALU = mybir.AluOpType
ACTF = mybir.ActivationFunctionType

TAU_SCALE = float(2.0 * np.log(MAX_BLUR / MIN_BLUR))
TAU_BIAS = float(np.log(0.5 * MIN_BLUR * MIN_BLUR))
HALF_LN999 = float(0.5 * np.log(1.0 - MIN_SCALE))

N_HOIST = 6  # x DMAs issued ahead of the slot stream


def _proc_order(cfg):
    """Interleave heavy (s>128) and light slots; start and end light."""
    n_b = len(cfg)
    heavy = [b for b in range(n_b) if cfg[b] > H]  # dealt order: heaviest first
    light = [b for b in range(n_b) if cfg[b] <= H][::-1]  # sparsest first
    if not light:
        return heavy
    order = [light[0]]
    li = 1
    for h in heavy:
        order.append(h)
        if li < len(light):
            order.append(light[li])
            li += 1
    order += light[li:]
    return order


def build_nc(cfg):
    """cfg: per-slot axis cutoff in dealt order, in {32,64,...,256}."""
    n_b = len(cfg)
    procorder = _proc_order(cfg)

    nc = bacc.Bacc(
        "TRN2",
        target_bir_lowering=False,
        debug=False,
        num_devices=NCORES,
    )
    x_d = nc.dram_tensor("x", [n_b, H, CH * W_IMG], BF16, kind="ExternalInput").ap()
    ctp_d = nc.dram_tensor("ctp", [H, 2 * N], BF16, kind="ExternalInput").ap()
    cmt_d = nc.dram_tensor("cmt", [H, 2 * N], BF16, kind="ExternalInput").ap()
    cm_d = nc.dram_tensor("cm", [H, 2 * N], BF16, kind="ExternalInput").ap()
    cm4_d = nc.dram_tensor("cm4", [H, 2 * N], BF16, kind="ExternalInput").ap()
    t128_d = nc.dram_tensor("t128", [H, n_b], F32, kind="ExternalInput").ap()
    f2t_d = nc.dram_tensor("f2t", [H, 2], F32, kind="ExternalInput").ap()
    y_d = nc.dram_tensor("y", [n_b, H, CH * W_IMG], BF16, kind="ExternalOutput").ap()

    # greedy ACT/DVE eviction balancer (ns estimates incl. fixed overhead)
    load = {"act": 0.0, "dve": 0.0}

    def pick(cols):
        a = load["act"] + 1.05 * cols + 260
        v = load["dve"] + 1.00 * cols + 180
        if a <= v:
            load["act"] = a
            return "act"
        load["dve"] = v
        return "dve"

    with tile.TileContext(nc) as tc, ExitStack() as ctx:
        cpool = ctx.enter_context(tc.tile_pool(name="consts", bufs=1))
        wpool = ctx.enter_context(tc.tile_pool(name="work", bufs=2))
        ppool = ctx.enter_context(tc.tile_pool(name="psum", bufs=2, space="PSUM"))

        def evict(dst, src, eng):
            if eng == "act":
                nc.scalar.copy(dst, src)
            else:
                nc.vector.tensor_copy(dst, src)

        # ---- constants; stage-1 needs only ctp + x, so those DMA first ----
        t128_sb = cpool.tile([H, n_b], F32, tag="t128", name="t128_sb")
        nc.sync.dma_start(t128_sb, t128_d)
        f2t_sb = cpool.tile([H, 2], F32, tag="f2t", name="f2t_sb")
        nc.sync.dma_start(f2t_sb, f2t_d)
        ctp = cpool.tile([H, 2 * N], BF16, tag="ctp", name="ctp")
        nc.sync.dma_start(ctp, ctp_d)

        xt = {}
        for b in procorder[:2]:
            xs = wpool.tile([H, CH * W_IMG], BF16, tag="x", bufs=N_HOIST + 2, name=f"x_{b}")
            nc.sync.dma_start(xs, x_d[b])
            xt[b] = xs

        cmt = cpool.tile([H, 2 * N], BF16, tag="cmt", name="cmt")
        nc.sync.dma_start(cmt, cmt_d)
        cm = cpool.tile([H, 2 * N], BF16, tag="cm", name="cm")
        nc.sync.dma_start(cm, cm_d)
        cm4 = cpool.tile([H, 2 * N], BF16, tag="cm4", name="cm4")
        nc.sync.dma_start(cm4, cm4_d)

        for b in procorder[2:N_HOIST]:
            xs = wpool.tile([H, CH * W_IMG], BF16, tag="x", bufs=N_HOIST + 2, name=f"x_{b}")
            nc.sync.dma_start(xs, x_d[b])
            xt[b] = xs

        # ---- PE warmup: junk matmuls with no data deps (HAM ramp) ----
        wtile = cpool.tile([H, H], BF16, tag="warm", name="warm")
        nc.vector.memset(wtile, 0.0)
        pj = ppool.tile([H, H], F32, tag="ps2", name="pjunk")
        for _ in range(12):
            nc.tensor.matmul(pj, wtile, wtile, start=True, stop=True)

        # ---- blur schedule ([p,1] scale APs for the separable mask) ----
        tbias128 = cpool.tile([H, 1], F32, tag="tbias128", name="tbias128")
        nc.vector.memset(tbias128, TAU_BIAS)
        hbias128 = cpool.tile([H, 1], F32, tag="hbias128", name="hbias128")
        nc.vector.memset(hbias128, HALF_LN999)
        tau128 = cpool.tile([H, n_b], F32, tag="tau128", name="tau128")
        nc.scalar.activation(tau128, t128_sb, ACTF.Exp, bias=tbias128, scale=TAU_SCALE)
        ntau128 = cpool.tile([H, n_b], F32, tag="ntau128", name="ntau128")
        nc.vector.tensor_scalar_mul(ntau128, tau128, -1.0)

        def get_x(b):
            if b in xt:
                return xt[b]
            xs = wpool.tile([H, CH * W_IMG], BF16, tag="x", bufs=N_HOIST + 2, name=f"x_{b}")
            nc.sync.dma_start(xs, x_d[b])
            return xs

        def emit_s12(b):
            s = cfg[b]
            kb = 1 if s <= H else 2
            kw = [min(s, H)] if kb == 1 else [H, s - H]
            xs = get_x(b)
            # separable mask: u carries sqrt(0.999) via the exp bias
            uT = wpool.tile([H, 2], F32, tag="ut", bufs=4, name=f"ut_{b}")
            nc.scalar.activation(
                uT, f2t_sb, ACTF.Exp, bias=hbias128, scale=ntau128[:, b : b + 1]
            )
            load["act"] += 200
            cmu = wpool.tile([H, kb * N], BF16, tag="cmu", bufs=3, name=f"cmu_{b}")
            cm4u = wpool.tile([H, kb * N], BF16, tag="cm4u", bufs=3, name=f"cm4u_{b}")
            for nn in range(kb):
                nc.vector.tensor_scalar_mul(
                    cmu[0 : kw[nn], nn * N : (nn + 1) * N],
                    cm[0 : kw[nn], nn * N : (nn + 1) * N],
                    uT[0 : kw[nn], nn : nn + 1],
                )
                nc.vector.tensor_scalar_mul(
                    cm4u[0 : kw[nn], nn * N : (nn + 1) * N],
                    cm4[0 : kw[nn], nn * N : (nn + 1) * N],
                    uT[0 : kw[nn], nn : nn + 1],
                )
                load["dve"] += 2 * (0.35 * N + 200)
            s1, s2 = {}, {}
            # stage 1: S1 = (C X)^T -> [w, k<s]; m = w-half, r = row parity
            for j in range(CH):
                p1 = ppool.tile([H, 2 * s], F32, tag="ps1", name=f"p1_{b}_{j}")
                for m in range(2):
                    for r in range(2):
                        nc.tensor.matmul(
                            p1[:, m * s : (m + 1) * s],
                            xs[:, j * W_IMG + r * N + m * H : j * W_IMG + r * N + m * H + H],
                            ctp[:, r * N : r * N + s],
                            start=(r == 0),
                            stop=(r == 1),
                        )
                t1 = wpool.tile([H, 2 * s], BF16, tag=f"s1x{j}", bufs=3, name=f"s1_{b}_{j}")
                evict(t1, p1, pick(2 * s))
                s1[j] = t1
            # stage 2: S2T = (C X C^T)^T -> [n<s, k<s]
            for j in range(CH):
                p2 = ppool.tile([H, kb * s], F32, tag="ps2", name=f"p2_{b}_{j}")
                for m2 in range(kb):
                    for ww in range(2):
                        nc.tensor.matmul(
                            p2[0 : kw[m2], m2 * s : m2 * s + s],
                            cmt[:, ww * N + m2 * H : ww * N + m2 * H + kw[m2]],
                            s1[j][:, ww * s : (ww + 1) * s],
                            start=(ww == 0),
                            stop=(ww == 1),
                        )
                t2 = wpool.tile([H, kb * s], BF16, tag=f"s2x{j}", bufs=3, name=f"s2_{b}_{j}")
                eng = pick(kb * s)
                for m2 in range(kb):
                    sl = (slice(0, kw[m2]), slice(m2 * s, m2 * s + s))
                    evict(t2[sl], p2[sl], eng)
                s2[j] = t2
            return dict(s=s, kb=kb, kw=kw, xs=xs, s2=s2, cmu=cmu, cm4u=cm4u)

        def emit_s34(b, st):
            s, kb, kw, xs, s2 = st["s"], st["kb"], st["kw"], st["xs"], st["s2"]
            cmu, cm4u = st["cmu"], st["cm4u"]
            s3 = {}
            # stage 3: S3 = S2 @ (u*C) -> [k<s, h]
            for j in range(CH):
                p3 = ppool.tile([H, kb * N], F32, tag="ps3", name=f"p3_{b}_{j}")
                for m3 in range(kb):
                    for nn in range(kb):
                        nc.tensor.matmul(
                            p3[0 : kw[m3], m3 * N : m3 * N + N],
                            s2[j][0 : kw[nn], nn * s + m3 * H : nn * s + m3 * H + kw[m3]],
                            cmu[0 : kw[nn], nn * N : nn * N + N],
                            start=(nn == 0),
                            stop=(nn == kb - 1),
                        )
                t3 = wpool.tile([H, kb * N], BF16, tag=f"s3x{j}", bufs=3, name=f"s3_{b}_{j}")
                eng = pick(kb * N)
                for m3 in range(kb):
                    sl = (slice(0, kw[m3]), slice(m3 * N, m3 * N + N))
                    evict(t3[sl], p3[sl], eng)
                s3[j] = t3
            # stage 4: Z = (u*C)^T @ S3 in row-pair layout; eviction fused
            o = wpool.tile([H, CH * W_IMG], BF16, tag="o", bufs=3, name=f"o_{b}")
            for j in range(CH):
                p4 = ppool.tile([H, 2 * N], F32, tag="ps4", name=f"p4_{b}_{j}")
                for m in range(2):
                    for kp in range(kb):
                        nc.tensor.matmul(
                            p4[:, m * N : (m + 1) * N],
                            cm4u[0 : kw[kp], kp * N + m * H : kp * N + m * H + H],
                            s3[j][0 : kw[kp], kp * N : kp * N + N],
                            start=(kp == 0),
                            stop=(kp == kb - 1),
                        )
                osl = o[:, j * W_IMG : (j + 1) * W_IMG]
                if pick(2 * N) == "dve":
                    nc.vector.scalar_tensor_tensor(
                        osl, xs[:, j * W_IMG : (j + 1) * W_IMG], MIN_SCALE, p4,
                        op0=ALU.mult, op1=ALU.add,
                    )
                else:
                    nc.scalar.copy(osl, p4)  # drops 0.001*X (~1e-3)
            nc.sync.dma_start(y_d[b], o)

        # 1-slot-lag software pipeline: stages 1-2 of slot i+1 are emitted
        # (and thus scheduled) ahead of stages 3-4 of slot i
        prev = None
        for b in procorder:
            st = emit_s12(b)
            if prev is not None:
                emit_s34(*prev)
            prev = (b, st)
        emit_s34(*prev)

    nc.compile()
    return nc


def host_constants():
    n = np.arange(N, dtype=np.float64)
    C = np.cos(np.pi * (n[None, :] + 0.5) * n[:, None] / N)
    scale = np.where(n[:, None] == 0, np.sqrt(1.0 / N), np.sqrt(2.0 / N))
    C = (C * scale).astype(np.float32)
    A = np.ascontiguousarray(C.T)  # A[h, k] = C[k, h]
    # ctp[p, r*N+k] = C[k, 2p+r]          (stage-1 rhs, rows even/odd split)
    ctp = A.reshape(H, 2, N).reshape(H, 2 * N)
    # cmt[p, ww*N+n] = C[n, ww*128+p]     (stage-2 weights, natural w-halves)
    cmt = A.reshape(2, H, N).transpose(1, 0, 2).reshape(H, 2 * N)
    # cm[p, nn*N+h] = C[nn*128+p, h]      (stage-3 rhs, natural rows)
    cm = C.reshape(2, H, N).transpose(1, 0, 2).reshape(H, 2 * N)
    # cm4[p, kp*N+m*H+w] = C[kp*128+p, 2w+m]  (stage-4 weights, cols split)
    cm4 = C.reshape(2, H, H, 2).transpose(1, 0, 3, 2).reshape(H, 2 * N)
    f = (np.pi * np.arange(N) / N).astype(np.float32)
    f2t = np.ascontiguousarray((f * f).reshape(2, H).T)
    return (
        np.ascontiguousarray(ctp.astype(BF16_NP)),
        np.ascontiguousarray(cmt.astype(BF16_NP)),
        np.ascontiguousarray(cm.astype(BF16_NP)),
        np.ascontiguousarray(cm4.astype(BF16_NP)),
        f2t,
    )


def s_of_t(t):
    """Per-item axis cutoff: smallest s (with safety margin) such that
    every kept fade entry (>=0.01) has both indices < s."""
    t64 = np.asarray(t, dtype=np.float64)
    sigma = np.exp(np.log(MIN_BLUR) * (1 - t64) + np.log(MAX_BLUR) * t64)
    tau = sigma * sigma / 2.0
    lim = np.log(100.0) / tau  # keep (i,j) with f_i^2+f_j^2 <= lim
    imax = np.floor(N * np.sqrt(lim) / np.pi).astype(np.int64)
    return np.minimum(imax + 3, N).astype(np.int64)


def interleave(img):
    """[3,256,256] fp32 -> [128, 3*512] row-pair layout."""
    return img.reshape(CH, H, 2, N).transpose(1, 0, 2, 3).reshape(H, CH * W_IMG)


def deinterleave(arr):
    """[128, 3*512] -> [3,256,256]."""
    return arr.reshape(H, CH, 2, N).transpose(1, 0, 2, 3).reshape(CH, N, N)


_CACHE = {}


def _get_nc(cfg):
    if cfg not in _CACHE:
        _CACHE[cfg] = build_nc(cfg)
    return _CACHE[cfg]


def _run(x, t, trace=False, tmpdir=None):
    x = np.ascontiguousarray(np.asarray(x, dtype=np.float32))
    t = np.asarray(t, dtype=np.float32)
    assert x.shape == (B, CH, N, N) and t.shape == (B,)

    s_item = s_of_t(t)
    # sort densest first, deal round-robin: slot b of core c gets item
    # order[b*8+c]; slot config = group max (sorted -> first of group)
    order = np.argsort(-s_item, kind="stable")
    cfg = tuple(
        int(min(N, ((int(s_item[order[b * NCORES]]) + 31) // 32) * 32))
        for b in range(BPC)
    )
    nc = _get_nc(cfg)

    ctp, cmt, cm, cm4, f2t = host_constants()
    in_maps = []
    for c in range(NCORES):
        items = order[np.arange(BPC) * NCORES + c]  # slot b -> batch index
        x16 = np.empty((BPC, H, CH * W_IMG), BF16_NP)
        for b in range(BPC):
            x16[b] = interleave(x[items[b]]).astype(BF16_NP)
        tc_ = np.ascontiguousarray(t[items].reshape(1, BPC))
        in_maps.append(
            {
                "x": x16,
                "ctp": ctp,
                "cmt": cmt,
                "cm": cm,
                "cm4": cm4,
                "t128": np.ascontiguousarray(np.broadcast_to(tc_, (H, BPC))),
                "f2t": f2t,
            }
        )
    res = run_bass_kernel_spmd(
        nc, in_maps, core_ids=list(range(NCORES)), trace=trace, tmpdir=tmpdir
    )
    out = np.empty_like(x)
    for c in range(NCORES):
        items = order[np.arange(BPC) * NCORES + c]
        y = res.results[c]["y"].astype(np.float32)
        for b in range(BPC):
            out[items[b]] = deinterleave(y[b])
    return out, res


def kernel(x, t):
    out, _ = _run(x, t)
    return out


def kernel_with_profile(x, t, tmpdir=None):
    out, res = _run(x, t, trace=True, tmpdir=tmpdir)
    return out, res
